# revision 1
# baseline (speedup 1.0000x reference)
"""Trainium2 Bass kernel: 6-layer causal transformer binary classifier.

Data-parallel over batch: B=8 rows -> 8 NeuronCores, one full forward per core.
Activations kept transposed ([H, S], H on partitions) so every matmul contracts
over the partition dim without runtime transposes (embedding is transposed once
via the PE). fp32r matmuls (full PE rate) for projections/W1; bf16 for the
attention path and Wo/W2. LayerNorm stats via ones-matmul partition reductions.
Causal structure skips fully-masked score tiles; the last layer only computes
queries/FFN for the final 128-token tile (only token S-1 feeds the classifier).
"""

import numpy as np
import ml_dtypes

import concourse.bass as bass
import concourse.mybir as mybir
import concourse.tile as tile
from concourse import bacc
from concourse.bass_utils import run_bass_kernel_spmd

F32 = mybir.dt.float32
F32R = mybir.dt.float32r
BF16 = mybir.dt.bfloat16
I32 = mybir.dt.int32

L_FULL, B_FULL, S_FULL, H, V = 6, 8, 2048, 768, 32000
FF = 4 * H
EPS = 1e-5
P = 128
HT = H // P          # 6 H-tiles
FT = FF // P         # 24 FF-tiles

# ppp column layout (per-partition params, [128, 72] per layer)
_BQ, _BK, _BO, _L1G, _L1B, _L2G, _L2B, _B2, _B1 = 0, 6, 12, 18, 24, 30, 36, 42, 48
_PPPW = 48 + FT

AF = mybir.ActivationFunctionType
OP = mybir.AluOpType


def _chunks(total, width):
    out = []
    c = 0
    while c < total:
        w = min(width, total - c)
        out.append((c, w))
        c += w
    return out


def build_nc(S, L, q_last=True):
    NT = S // P
    nc = bacc.Bacc("TRN2")

    emb_x = nc.declare_dram_parameter("tok_emb", [V, H], F32, isOutput=False)
    pos_x = nc.declare_dram_parameter("pos", [S, H], F32, isOutput=False)
    ids_x = nc.declare_dram_parameter("ids32", [NT, P, 1], I32, isOutput=False)
    ident_x = nc.declare_dram_parameter("ident", [P, P], F32, isOutput=False)
    cmask_x = nc.declare_dram_parameter("cmask", [P, P], BF16, isOutput=False)
    wqkv_x = nc.declare_dram_parameter("wqkv", [L, 3, HT, P, H], BF16, isOutput=False)
    wo_x = nc.declare_dram_parameter("wo_p", [L, HT, P, H], BF16, isOutput=False)
    w1_x = nc.declare_dram_parameter("w1_p", [L, FT, P, H], BF16, isOutput=False)
    w2_x = nc.declare_dram_parameter("w2_p", [L, HT, P, FF], BF16, isOutput=False)
    ppp_x = nc.declare_dram_parameter("ppp", [L, P, _PPPW], F32, isOutput=False)
    bv_x = nc.declare_dram_parameter("bv_raw", [L, H], F32, isOutput=False)
    fpp_x = nc.declare_dram_parameter("fpp", [P, 18], F32, isOutput=False)
    clsb_x = nc.declare_dram_parameter("clsb", [1, 1], F32, isOutput=False)
    out_x = nc.declare_dram_parameter("out", [1, 1], F32, isOutput=True)

    scale = 1.0 / float(np.sqrt(H))

    with tile.TileContext(nc) as tc:
        with tc.tile_pool(name="persist", bufs=1) as pp:
            ident = pp.tile([P, P], F32, tag="ident")
            nc.sync.dma_start(out=ident, in_=ident_x[:])
            cmask = pp.tile([P, P], BF16, tag="cmask")
            nc.sync.dma_start(out=cmask, in_=cmask_x[:])
            ones_f = pp.tile([P, P], F32, tag="ones_f")
            nc.vector.memset(ones_f, 1.0)
            ones_r = pp.tile([P, P], F32R, tag="ones_r")
            nc.vector.tensor_copy(ones_r, ones_f)
            ones_b = pp.tile([P, P], BF16, tag="ones_b")
            nc.vector.memset(ones_b, 1.0)
            eps_t = pp.tile([P, 1], F32, tag="eps")
            nc.vector.memset(eps_t, EPS)
            fpp = pp.tile([P, 18], F32, tag="fpp")
            nc.sync.dma_start(out=fpp, in_=fpp_x[:])

            xT = pp.tile([P, HT, S], F32R, tag="xT")
            xR = xT.bitcast(F32)  # read view for DVE

            # ---- embedding: gather + pos, then PE-transpose into xT ----
            with tc.tile_pool(name="emb", bufs=1) as ep, \
                 tc.tile_pool(name="embp", bufs=1, space="PSUM") as epp:
                for tt in range(NT):
                    ids_t = ep.tile([P, 1], I32, tag="ids", bufs=2)
                    nc.sync.dma_start(out=ids_t, in_=ids_x[tt])
                    xn = ep.tile([P, H], F32, tag="xn", bufs=3)
                    nc.gpsimd.indirect_dma_start(
                        out=xn[:], out_offset=None, in_=emb_x[:],
                        in_offset=bass.IndirectOffsetOnAxis(ap=ids_t[:, :1], axis=0))
                    pos_t = ep.tile([P, H], F32, tag="pos", bufs=2)
                    nc.sync.dma_start(out=pos_t, in_=pos_x[tt * P:(tt + 1) * P, :])
                    nc.vector.tensor_add(xn, xn, pos_t)
                    for c in range(HT):
                        trp = epp.tile([P, P], F32, tag="tr", bufs=4, space="PSUM")
                        nc.tensor.transpose(out=trp[:], in_=xn[:, c * P:(c + 1) * P],
                                            identity=ident[:])
                        nc.vector.tensor_copy(xT[:, c, tt * P:(tt + 1) * P], trp)

            # ---- layer-norm (transposed layout) helper ----
            def ln_T(sb, ps, c0, w, out_tile, gcol, bcol, ptag, stat_w, pbufs):
                s1 = ps.tile([P, stat_w], F32, tag=ptag, bufs=pbufs, space="PSUM")
                for c in range(HT):
                    for (n0, nw) in _chunks(w, 512):
                        nc.tensor.matmul(s1[:, n0:n0 + nw], ones_r,
                                         xT[:, c, c0 + n0:c0 + n0 + nw],
                                         start=(c == 0), stop=(c == HT - 1))
                s2 = ps.tile([P, stat_w], F32, tag=ptag, bufs=pbufs, space="PSUM")
                for c in range(HT):
                    sq = sb.tile([P, stat_w], F32R, tag="sq", bufs=2)
                    nc.vector.tensor_mul(sq[:, :w], xR[:, c, c0:c0 + w], xR[:, c, c0:c0 + w])
                    for (n0, nw) in _chunks(w, 512):
                        nc.tensor.matmul(s2[:, n0:n0 + nw], ones_r, sq[:, n0:n0 + nw],
                                         start=(c == 0), stop=(c == HT - 1))
                # rstd = rsqrt(E[x^2] - mean^2 + eps) via one ACT op; gcol is
                # host-negated so apply can use (mean - x) without a negate.
                mn = sb.tile([P, stat_w], F32, tag="lnmn", bufs=2)
                nc.vector.tensor_scalar_mul(mn[:, :w], s1[:, :w], 1.0 / H)
                nc.vector.tensor_mul(mn[:, :w], mn[:, :w], mn[:, :w])
                rstd = sb.tile([P, stat_w], F32, tag="lnrstd", bufs=2)
                nc.vector.scalar_tensor_tensor(
                    out=rstd[:, :w], in0=s2[:, :w], scalar=1.0 / H,
                    in1=mn[:, :w], op0=OP.mult, op1=OP.subtract)
                nc.scalar.activation(out=rstd[:, :w], in_=rstd[:, :w],
                                     func=AF.Abs_reciprocal_sqrt, bias=eps_t[:])
                for c in range(HT):
                    t = sb.tile([P, stat_w], F32, tag="lnt", bufs=2)
                    nc.vector.scalar_tensor_tensor(
                        out=t[:, :w], in0=s1[:, :w], scalar=1.0 / H,
                        in1=xR[:, c, c0:c0 + w], op0=OP.mult, op1=OP.subtract)
                    nc.vector.tensor_mul(t[:, :w], t[:, :w], rstd[:, :w])
                    nc.vector.tensor_scalar(
                        out=out_tile[:, c, :w], in0=t[:, :w],
                        scalar1=gcol[:, c:c + 1], scalar2=bcol[:, c:c + 1],
                        op0=OP.mult, op1=OP.add)

            # ---- transformer layers ----
            for l in range(L):
                last = q_last and (l == L - 1)

                # ===== attention =====
                with tc.tile_pool(name=f"at{l}", bufs=1) as sb:
                    ppp = sb.tile([P, _PPPW], F32, tag="ppp")
                    nc.sync.dma_start(out=ppp, in_=ppp_x[l])
                    bvb = sb.tile([P, H], F32, tag="bvb")
                    nc.sync.dma_start(out=bvb, in_=bv_x[l].partition_broadcast(P))

                    qt = sb.tile([P, HT, S], BF16, tag="qt")
                    kt_ = sb.tile([P, HT, S], BF16, tag="kt")
                    vn = sb.tile([P, NT, H], BF16, tag="vn")

                    qkv_ps = tc.alloc_tile_pool(name=f"atp{l}", bufs=1, space="PSUM")
                    ps = qkv_ps
                    for (c0, cw) in _chunks(S, 512):
                        hc = sb.tile([P, HT, 512], BF16, tag="hc", bufs=2)
                        ln_T(sb, ps, c0, cw, hc,
                             ppp[:, _L1G:_L1G + HT], ppp[:, _L1B:_L1B + HT],
                             "qp", 512, 3)
                        need_q = (not last) or (c0 + cw > S - P)
                        for i, (outt, bcol) in enumerate(
                                ((qt, _BQ), (kt_, _BK), (vn, None))):
                            if i == 0 and not need_q:
                                continue
                            w_t = sb.tile([P, HT, H], BF16, tag="wres", bufs=2)
                            nc.sync.dma_start(
                                out=w_t, in_=wqkv_x[l, i].rearrange("c p j -> p c j"))
                            if i < 2:
                                for m in range(HT):
                                    pj = ps.tile([P, 512], F32, tag="qp", bufs=3, space="PSUM")
                                    for k in range(HT):
                                        nc.tensor.matmul(
                                            pj[:, :cw], w_t[:, k, m * P:(m + 1) * P],
                                            hc[:, k, :cw],
                                            start=(k == 0), stop=(k == HT - 1))
                                    if i == 0:
                                        nc.vector.tensor_scalar(
                                            out=outt[:, m, c0:c0 + cw], in0=pj[:, :cw],
                                            scalar1=ppp[:, bcol + m:bcol + m + 1],
                                            scalar2=scale, op0=OP.add, op1=OP.mult)
                                    else:
                                        nc.vector.tensor_scalar_add(
                                            outt[:, m, c0:c0 + cw], pj[:, :cw],
                                            ppp[:, bcol + m:bcol + m + 1])
                            else:
                                for t in range(cw // P):
                                    tt = (c0 // P) + t
                                    pv = ps.tile([P, H], F32, tag="vp", bufs=2,
                                                 space="PSUM")
                                    for (j0, jw) in _chunks(H, 512):
                                        for k in range(HT):
                                            nc.tensor.matmul(
                                                pv[:, j0:j0 + jw],
                                                hc[:, k, t * P:(t + 1) * P],
                                                w_t[:, k, j0:j0 + jw],
                                                start=(k == 0), stop=(k == HT - 1))
                                    nc.vector.tensor_add(vn[:, tt, :], pv, bvb)

                    qkv_ps.release()
                    att_ps = tc.alloc_tile_pool(name=f"atq{l}", bufs=1, space="PSUM")
                    ps = att_ps

                    # attention blocks
                    q_blocks = [(S - P, P)] if last else _chunks(S, 512)
                    for (q0, qw) in q_blocks:
                        ktmax = (q0 + qw - 1) // P
                        attnT = sb.tile([P, NT, 512], BF16, tag="attnT")
                        for kt in range(ktmax + 1):
                            scp = ps.tile([P, 512], F32, tag="sc", bufs=2, space="PSUM")
                            for c in range(HT):
                                nc.tensor.matmul(
                                    scp[:, :qw], kt_[:, c, kt * P:(kt + 1) * P],
                                    qt[:, c, q0:q0 + qw],
                                    start=(c == 0), stop=(c == HT - 1))
                            lo = kt * P - q0
                            if lo + P <= 0:
                                nc.scalar.activation(out=attnT[:, kt, :qw],
                                                     in_=scp[:, :qw], func=AF.Exp)
                            else:
                                if lo > 0:
                                    nc.vector.memset(attnT[:, kt, 0:lo], 0.0)
                                d0 = max(lo, 0)
                                d1 = min(lo + P, qw)
                                dt_ = sb.tile([P, P], BF16, tag="dtmp", bufs=2)
                                nc.scalar.activation(out=dt_[:, :d1 - d0],
                                                     in_=scp[:, d0:d1], func=AF.Exp)
                                nc.vector.tensor_mul(attnT[:, kt, d0:d1],
                                                     dt_[:, :d1 - d0],
                                                     cmask[:, d0 - lo:d1 - lo])
                                if d1 < qw:
                                    nc.scalar.activation(out=attnT[:, kt, d1:qw],
                                                         in_=scp[:, d1:qw], func=AF.Exp)
                        dnp = ps.tile([P, 512], F32, tag="dn", bufs=2, space="PSUM")
                        for kt in range(ktmax + 1):
                            nc.tensor.matmul(dnp[:, :qw], ones_b, attnT[:, kt, :qw],
                                             start=(kt == 0), stop=(kt == ktmax))
                        # 1/x via exp(-ln(x)) on ACT (DVE reciprocal is ~9 cyc/elem)
                        dnl = sb.tile([P, 512], F32, tag="dnl", bufs=2)
                        nc.scalar.activation(out=dnl[:, :qw], in_=dnp[:, :qw],
                                             func=AF.Ln)
                        dn = sb.tile([P, 512], F32, tag="dns", bufs=2)
                        nc.scalar.activation(out=dn[:, :qw], in_=dnl[:, :qw],
                                             func=AF.Exp, scale=-1.0)
                        ot = sb.tile([P, HT, 512], BF16, tag="ot")
                        for m in range(HT):
                            avp = ps.tile([P, 512], F32, tag="av", bufs=2, space="PSUM")
                            for kt in range(ktmax + 1):
                                nc.tensor.matmul(
                                    avp[:, :qw], vn[:, kt, m * P:(m + 1) * P],
                                    attnT[:, kt, :qw],
                                    start=(kt == 0), stop=(kt == ktmax))
                            nc.vector.tensor_mul(ot[:, m, :qw], avp[:, :qw], dn[:, :qw])
                        for m2 in range(HT):
                            woc = sb.tile([P, H], BF16, tag="woc", bufs=2)
                            nc.sync.dma_start(out=woc, in_=wo_x[l, m2])
                            wop = ps.tile([P, 512], F32, tag="wo", bufs=2, space="PSUM")
                            for k in range(HT):
                                nc.tensor.matmul(wop[:, :qw], woc[:, k * P:(k + 1) * P],
                                                 ot[:, k, :qw],
                                                 start=(k == 0), stop=(k == HT - 1))
                            xsl = xT[:, m2, q0:q0 + qw]
                            nc.vector.tensor_add(xsl, wop[:, :qw], xR[:, m2, q0:q0 + qw])
                            nc.vector.tensor_scalar_add(
                                xsl, xR[:, m2, q0:q0 + qw],
                                ppp[:, _BO + m2:_BO + m2 + 1])
                    att_ps.release()

                # ===== FFN =====
                with tc.tile_pool(name=f"ff{l}", bufs=1) as sb, \
                     tc.tile_pool(name=f"ffp{l}", bufs=1, space="PSUM") as ps:
                    ppp = sb.tile([P, _PPPW], F32, tag="ppp")
                    nc.sync.dma_start(out=ppp, in_=ppp_x[l])
                    f_chunks = [(S - P, P)] if last else _chunks(S, 1024)
                    for (c0, cw) in f_chunks:
                        h2c = sb.tile([P, HT, 1024], BF16, tag="h2c", bufs=2)
                        ln_T(sb, ps, c0, cw, h2c,
                             ppp[:, _L2G:_L2G + HT], ppp[:, _L2B:_L2B + HT],
                             "fp", 1024, 4)
                        h2r = h2c
                        g1 = sb.tile([P, FT, 1024], BF16, tag="g1")
                        for m in range(FT):
                            w1c = sb.tile([P, H], BF16, tag="w1c", bufs=2)
                            nc.sync.dma_start(out=w1c, in_=w1_x[l, m])
                            f1p = ps.tile([P, 1024], F32, tag="fp", bufs=4, space="PSUM")
                            for k in range(HT):
                                for (n0, nw) in _chunks(cw, 512):
                                    nc.tensor.matmul(
                                        f1p[:, n0:n0 + nw], w1c[:, k * P:(k + 1) * P],
                                        h2r[:, k, n0:n0 + nw],
                                        start=(k == 0), stop=(k == HT - 1))
                            nc.scalar.activation(out=g1[:, m, :cw], in_=f1p[:, :cw],
                                                 func=AF.Gelu,
                                                 bias=ppp[:, _B1 + m:_B1 + m + 1])
                        for m2 in range(HT):
                            w2c = sb.tile([P, FF], BF16, tag="w2c", bufs=2)
                            nc.sync.dma_start(out=w2c, in_=w2_x[l, m2])
                            f2p = ps.tile([P, 1024], F32, tag="fp", bufs=4, space="PSUM")
                            for k in range(FT):
                                for (n0, nw) in _chunks(cw, 512):
                                    nc.tensor.matmul(
                                        f2p[:, n0:n0 + nw], w2c[:, k * P:(k + 1) * P],
                                        g1[:, k, n0:n0 + nw],
                                        start=(k == 0), stop=(k == FT - 1))
                            xsl = xT[:, m2, c0:c0 + cw]
                            nc.vector.tensor_add(xsl, f2p[:, :cw], xR[:, m2, c0:c0 + cw])
                            nc.vector.tensor_scalar_add(
                                xsl, xR[:, m2, c0:c0 + cw],
                                ppp[:, _B2 + m2:_B2 + m2 + 1])

            # ---- final LN on last column + classifier ----
            with tc.tile_pool(name="head", bufs=1) as sb, \
                 tc.tile_pool(name="headp", bufs=1, space="PSUM") as ps:
                col = S - 1
                s1 = ps.tile([P, 1], F32, tag="hp", bufs=2, space="PSUM")
                for c in range(HT):
                    nc.tensor.matmul(s1, ones_f, xR[:, c, col:col + 1],
                                     start=(c == 0), stop=(c == HT - 1))
                sqc = sb.tile([P, HT, 1], F32, tag="hsq")
                nc.vector.tensor_mul(sqc, xR[:, :, col:col + 1], xR[:, :, col:col + 1])
                s2 = ps.tile([P, 1], F32, tag="hp", bufs=2, space="PSUM")
                for c in range(HT):
                    nc.tensor.matmul(s2, ones_f, sqc[:, c, :],
                                     start=(c == 0), stop=(c == HT - 1))
                mean = sb.tile([P, 1], F32, tag="hmean")
                nc.vector.tensor_scalar_mul(mean, s1, 1.0 / H)
                rstd = sb.tile([P, 1], F32, tag="hrstd")
                nc.vector.tensor_scalar_mul(rstd, s2, 1.0 / H)
                m2_ = sb.tile([P, 1], F32, tag="hm2")
                nc.vector.tensor_mul(m2_, mean, mean)
                nc.vector.tensor_sub(rstd, rstd, m2_)
                nc.scalar.activation(out=rstd, in_=rstd, func=AF.Sqrt, bias=eps_t[:])
                nc.vector.reciprocal(rstd, rstd)
                nf = sb.tile([P, HT, 1], F32, tag="hnf")
                for c in range(HT):
                    t = sb.tile([P, 1], F32, tag="ht", bufs=2)
                    nc.vector.tensor_sub(t, xR[:, c, col:col + 1], mean)
                    nc.vector.tensor_mul(t, t, rstd)
                    nc.vector.tensor_scalar(
                        out=nf[:, c, :], in0=t,
                        scalar1=fpp[:, c:c + 1], scalar2=fpp[:, 6 + c:7 + c],
                        op0=OP.mult, op1=OP.add)
                lp = ps.tile([P, 1], F32, tag="hp", bufs=2, space="PSUM")
                for c in range(HT):
                    nc.tensor.matmul(lp[0:1, :], nf[:, c, :], fpp[:, 12 + c:13 + c],
                                     start=(c == 0), stop=(c == HT - 1))
                cb = sb.tile([P, 1], F32, tag="hcb")
                nc.sync.dma_start(out=cb[0:1, :], in_=clsb_x[:])
                oo = sb.tile([P, 1], F32, tag="hoo")
                nc.vector.tensor_add(oo[0:1, :], lp[0:1, :], cb[0:1, :])
                nc.sync.dma_start(out=out_x[:], in_=oo[0:1, :])

    nc.finalize()
    return nc


def _pack_host(inputs, S, L):
    """Shared (replicated) host-side packed arrays."""
    f32 = np.float32
    bf16 = ml_dtypes.bfloat16

    def npf(x):
        return np.asarray(x, dtype=f32)

    Wq, Wk, Wv = npf(inputs["Wq"]), npf(inputs["Wk"]), npf(inputs["Wv"])
    Wo, W1, W2 = npf(inputs["Wo"]), npf(inputs["W1"]), npf(inputs["W2"])

    wqkv = np.stack([Wq, Wk, Wv], axis=1).reshape(L, 3, HT, P, H).astype(bf16)
    wo_p = np.ascontiguousarray(
        Wo.reshape(L, HT, P, HT, P).transpose(0, 3, 2, 1, 4).reshape(L, HT, P, H)
    ).astype(bf16)
    w1_p = np.ascontiguousarray(
        W1.reshape(L, HT, P, FT, P).transpose(0, 3, 2, 1, 4).reshape(L, FT, P, H)
    ).astype(bf16)
    w2_p = np.ascontiguousarray(
        W2.reshape(L, FT, P, HT, P).transpose(0, 3, 2, 1, 4).reshape(L, HT, P, FF)
    ).astype(bf16)

    def pcol(v, n):  # [L, n*128] -> [L, 128, n]
        return np.ascontiguousarray(np.transpose(v.reshape(L, n, P), (0, 2, 1)))

    # ln gammas negated: the device LN applies (mean - x) * rstd * (-g) + b
    ppp = np.concatenate([
        pcol(npf(inputs["bq"]), HT), pcol(npf(inputs["bk"]), HT),
        pcol(npf(inputs["bo"]), HT),
        pcol(-npf(inputs["ln1_g"]), HT), pcol(npf(inputs["ln1_b"]), HT),
        pcol(-npf(inputs["ln2_g"]), HT), pcol(npf(inputs["ln2_b"]), HT),
        pcol(npf(inputs["b2"]), HT), pcol(npf(inputs["b1"]), FT),
    ], axis=2)
    assert ppp.shape == (L, P, _PPPW)

    fpp = np.concatenate([
        npf(inputs["fln_g"]).reshape(HT, P).T,
        npf(inputs["fln_b"]).reshape(HT, P).T,
        npf(inputs["cls_W"]).reshape(HT, P).T,
    ], axis=1)

    cm = (np.arange(P)[None, :] >= np.arange(P)[:, None])  # [k, q] valid q>=k

    return {
        "tok_emb": npf(inputs["tok_emb"]),
        "pos": npf(inputs["pos_emb"])[:S],
        "ident": np.eye(P, dtype=f32),
        "cmask": cm.astype(bf16),
        "wqkv": wqkv, "wo_p": wo_p, "w1_p": w1_p, "w2_p": w2_p,
        "ppp": ppp, "bv_raw": npf(inputs["bv"]),
        "fpp": np.ascontiguousarray(fpp),
        "clsb": npf(inputs["cls_b"]).reshape(1, 1),
    }


_NC_CACHE = {}


def run_model(inputs, S=S_FULL, L=L_FULL, B=B_FULL, q_last=True, trace=False):
    mask = np.asarray(inputs["attention_mask"])
    if not np.all(mask == 1):
        raise NotImplementedError("padded attention_mask not supported")

    shared = _pack_host(inputs, S, L)
    ids = np.asarray(inputs["input_ids"]).astype(np.int32)  # [B, S]
    in_maps = []
    for b in range(B):
        m = dict(shared)
        m["ids32"] = np.ascontiguousarray(ids[b].reshape(S // P, P, 1))
        in_maps.append(m)

    key = (S, L, q_last)
    if key not in _NC_CACHE:
        _NC_CACHE[key] = build_nc(S, L, q_last)
    nc = _NC_CACHE[key]

    res = run_bass_kernel_spmd(nc, in_maps, list(range(B)), trace=trace)
    out = np.stack([res.results[b]["out"].reshape(1) for b in range(B)], axis=0)
    return out.astype(np.float32), res


def kernel(**inputs) -> np.ndarray:
    out, _ = run_model(inputs, S=S_FULL, L=L_FULL, B=B_FULL)
    return out



# revision 2
# speedup vs baseline: 1.0168x; 1.0168x over previous
"""Trainium2 Bass kernel v2: 6-layer causal transformer binary classifier.

Data-parallel over batch: B=8 rows -> 8 NeuronCores. Activations transposed
([H, S], H on partitions). Residual stream in bf16. LN gamma/beta folded into
the following weights/biases on the host (k-bias dropped: softmax-row
invariant; v-bias folded into Wo bias via softmax normalization). All big
GEMMs bf16 with full-rate back-to-back streams (LDWEIGHTS hidden); attention
scores/AV in fp8 e4m3 with DoubleRow (2x PE rate). 1/sqrt(H) folded into the
softmax exp; exp scaled by 16 into fp8 range, cancels in normalization.
"""

import numpy as np
import ml_dtypes

import concourse.bass as bass
import concourse.mybir as mybir
import concourse.tile as tile
from concourse import bacc
from concourse.bass_utils import run_bass_kernel_spmd

F32 = mybir.dt.float32
BF16 = mybir.dt.bfloat16
FP8 = mybir.dt.float8e4
I32 = mybir.dt.int32
DRMODE = mybir.MatmulPerfMode.DoubleRow

L_FULL, B_FULL, S_FULL, H, V = 6, 8, 2048, 768, 32000
FF = 4 * H
EPS = 1e-5
P = 128
HT = H // P          # 6
FT = FF // P         # 24

# ppp column layout per layer: bq'(HT), bo'(HT), b2(HT), b1'(FT)
_BQ, _BO, _B2, _B1 = 0, 6, 12, 18
_PPPW = 18 + FT      # 42

AF = mybir.ActivationFunctionType
OP = mybir.AluOpType

ATTN_FP8 = True
LN16 = float(np.log(16.0))
SCALE = 1.0 / float(np.sqrt(H))


def _chunks(total, width):
    out = []
    c = 0
    while c < total:
        w = min(width, total - c)
        out.append((c, w))
        c += w
    return out


def build_nc(S, L, q_last=True):
    NT = S // P
    nc = bacc.Bacc("TRN2")

    emb_x = nc.declare_dram_parameter("tok_emb", [V, H], F32, isOutput=False)
    pos_x = nc.declare_dram_parameter("pos", [S, H], F32, isOutput=False)
    ids_x = nc.declare_dram_parameter("ids32", [NT, P, 1], I32, isOutput=False)
    ident_x = nc.declare_dram_parameter("ident", [P, P], F32, isOutput=False)
    cmask_x = nc.declare_dram_parameter("cmask", [P, P], BF16, isOutput=False)
    wqkv_x = nc.declare_dram_parameter("wqkv", [L, 3, HT, P, H], BF16, isOutput=False)
    wo_x = nc.declare_dram_parameter("wo_p", [L, HT, P, H], BF16, isOutput=False)
    w1_x = nc.declare_dram_parameter("w1_p", [L, HT, P, FF], BF16, isOutput=False)
    w2_x = nc.declare_dram_parameter("w2_p", [L, FT, P, H], BF16, isOutput=False)
    ppp_x = nc.declare_dram_parameter("ppp", [L, P, _PPPW], F32, isOutput=False)
    fpp_x = nc.declare_dram_parameter("fpp", [P, 18], F32, isOutput=False)
    clsb_x = nc.declare_dram_parameter("clsb", [1, 1], F32, isOutput=False)
    out_x = nc.declare_dram_parameter("out", [1, 1], F32, isOutput=True)

    qk_dt = FP8 if ATTN_FP8 else BF16

    with tile.TileContext(nc) as tc:
        with tc.tile_pool(name="persist", bufs=1) as pp:
            ident = pp.tile([P, P], F32, tag="ident")
            nc.sync.dma_start(out=ident, in_=ident_x[:])
            cmask = pp.tile([P, P], BF16, tag="cmask")
            nc.sync.dma_start(out=cmask, in_=cmask_x[:])
            ones_b = pp.tile([P, P], BF16, tag="ones_b")
            nc.vector.memset(ones_b, 1.0)
            ones_8 = pp.tile([P, 2, P], qk_dt, tag="ones_8")
            nc.vector.memset(ones_8, 1.0)
            eps_t = pp.tile([P, 1], F32, tag="eps")
            nc.vector.memset(eps_t, EPS)
            ln16_t = pp.tile([P, 1], F32, tag="ln16")
            nc.vector.memset(ln16_t, LN16 if ATTN_FP8 else 0.0)
            fpp = pp.tile([P, 18], F32, tag="fpp")
            nc.sync.dma_start(out=fpp, in_=fpp_x[:])

            xT = pp.tile([P, HT, S], BF16, tag="xT")
            qt8 = pp.tile([P, HT, S], qk_dt, tag="qt8")
            kt8 = pp.tile([P, HT, S], qk_dt, tag="kt8")
            vn8 = pp.tile([P, NT, H], qk_dt, tag="vn8")

            # ---- embedding: gather + pos, PE-transpose into xT (bf16) ----
            with tc.tile_pool(name="emb", bufs=1) as ep, \
                 tc.tile_pool(name="embp", bufs=1, space="PSUM") as epp:
                for tt in range(NT):
                    ids_t = ep.tile([P, 1], I32, tag="ids", bufs=4)
                    nc.sync.dma_start(out=ids_t, in_=ids_x[tt])
                    xn = ep.tile([P, H], F32, tag="xn", bufs=6)
                    nc.gpsimd.indirect_dma_start(
                        out=xn[:], out_offset=None, in_=emb_x[:],
                        in_offset=bass.IndirectOffsetOnAxis(ap=ids_t[:, :1], axis=0))
                    pos_t = ep.tile([P, H], F32, tag="pos", bufs=4)
                    nc.sync.dma_start(out=pos_t, in_=pos_x[tt * P:(tt + 1) * P, :])
                    nc.vector.tensor_add(xn, xn, pos_t)
                    for c in range(HT):
                        trp = epp.tile([P, P], F32, tag="tr", bufs=4, space="PSUM")
                        nc.tensor.transpose(out=trp[:], in_=xn[:, c * P:(c + 1) * P],
                                            identity=ident[:])
                        nc.vector.tensor_copy(xT[:, c, tt * P:(tt + 1) * P], trp)

            # ---- LN stats helper (emits PE matmuls + DVE math) ----
            def ln_stats(sb, ps, c0, w, ptag, pbufs):
                """returns (mn bf16 [P,w], rstd bf16 [P,w])"""
                s1 = ps.tile([P, 512], F32, tag=ptag, bufs=pbufs, name="s1",
                             space="PSUM")
                for c in range(HT):
                    nc.tensor.matmul(s1[:, :w], ones_b, xT[:, c, c0:c0 + w],
                                     start=(c == 0), stop=(c == HT - 1))
                s2 = ps.tile([P, 512], F32, tag=ptag, bufs=pbufs, name="s2",
                             space="PSUM")
                for c in range(HT):
                    sq = sb.tile([P, 512], BF16, tag="sq", bufs=1, name="sq")
                    nc.vector.tensor_mul(sq[:, :w], xT[:, c, c0:c0 + w],
                                         xT[:, c, c0:c0 + w])
                    nc.tensor.matmul(s2[:, :w], ones_b, sq[:, :w],
                                     start=(c == 0), stop=(c == HT - 1))
                mn = sb.tile([P, 512], BF16, tag="mn", bufs=2, name="mn")
                nc.vector.tensor_scalar_mul(mn[:, :w], s1[:, :w], 1.0 / H)
                m2 = sb.tile([P, 512], BF16, tag="lnt", bufs=1, name="m2")
                nc.vector.tensor_mul(m2[:, :w], mn[:, :w], mn[:, :w])
                rstd = sb.tile([P, 512], BF16, tag="rstd", bufs=2, name="rstd")
                rs32 = sb.tile([P, 512], BF16, tag="sq", bufs=1, name="rs32")
                nc.vector.scalar_tensor_tensor(
                    out=rs32[:, :w], in0=s2[:, :w], scalar=1.0 / H,
                    in1=m2[:, :w], op0=OP.mult, op1=OP.subtract)
                nc.scalar.activation(out=rstd[:, :w], in_=rs32[:, :w],
                                     func=AF.Abs_reciprocal_sqrt, bias=eps_t[:])
                return mn, rstd

            def ln_apply(sb, mn, rstd, c0, w, out_tile, out_off):
                for c in range(HT):
                    t = sb.tile([P, 512], BF16, tag="lnt", bufs=1, name="lnt")
                    nc.vector.tensor_sub(t[:, :w], xT[:, c, c0:c0 + w], mn[:, :w])
                    nc.vector.tensor_mul(out_tile[:, c, out_off:out_off + w],
                                         t[:, :w], rstd[:, :w])

            # ---- transformer layers ----
            gps = tc.alloc_tile_pool(name="gps", bufs=1, space="PSUM")
            cks = _chunks(S, 512)
            pend1 = {}

            def stats1(l_, ci):
                c0, cw = cks[ci]
                mn, rstd = ln_stats(pp, gps, c0, cw, "mm", 4)
                hc = pp.tile([P, HT, 512], BF16, tag="hc", bufs=2, name="hc1")
                ln_apply(pp, mn, rstd, c0, cw, hc, 0)
                pend1[(l_, ci)] = hc

            for l in range(L):
                last = q_last and (l == L - 1)

                lw_qkv = tc.alloc_tile_pool(name=f"wqkv{l}", bufs=1)
                wqkv_sb = lw_qkv.tile([P, 3, HT, H], BF16, tag="wqkv", name="wqkv")
                nc.sync.dma_start(
                    out=wqkv_sb, in_=wqkv_x[l].rearrange("i c p j -> p i c j"))
                lay = pp
                ppp = pp.tile([P, _PPPW], F32, tag="ppp", bufs=2, name=f"ppp{l}")
                nc.sync.dma_start(out=ppp, in_=ppp_x[l])

                # ===== QKV phase (stats pipelined two chunks ahead) =====
                for ci in (0, 1):
                    if ci < len(cks) and (l, ci) not in pend1:
                        stats1(l, ci)
                for ci, (c0, cw) in enumerate(cks):
                    hc = pend1.pop((l, ci))
                    # Q (skip unless needed)
                    if (not last) or (c0 + cw == S):
                        qo, qcw = (384, P) if last else (0, cw)
                        for m in range(HT):
                            pj = gps.tile([P, 512], F32, tag="mm", bufs=4,
                                          name="pjq", space="PSUM")
                            for k in range(HT):
                                nc.tensor.matmul(
                                    pj[:, :qcw], wqkv_sb[:, 0, k, m * P:(m + 1) * P],
                                    hc[:, k, qo:qo + qcw],
                                    start=(k == 0), stop=(k == HT - 1))
                            nc.vector.tensor_scalar_add(
                                qt8[:, m, c0 + qo:c0 + qo + qcw], pj[:, :qcw],
                                ppp[:, _BQ + m:_BQ + m + 1])
                    # K
                    for m in range(HT):
                        pj = gps.tile([P, 512], F32, tag="mm", bufs=4,
                                      name="pjk", space="PSUM")
                        for k in range(HT):
                            nc.tensor.matmul(
                                pj[:, :cw], wqkv_sb[:, 1, k, m * P:(m + 1) * P],
                                hc[:, k, :cw],
                                start=(k == 0), stop=(k == HT - 1))
                        nc.vector.tensor_copy(kt8[:, m, c0:c0 + cw], pj[:, :cw])
                    # V: stationary = hc token-tile, moving = weights
                    for t in range(cw // P):
                        tt = (c0 // P) + t
                        pv = gps.tile([P, 1024], F32, tag="vp", bufs=2,
                                      name="pv", space="PSUM")
                        for (j0, jw) in _chunks(H, 512):
                            for k in range(HT):
                                nc.tensor.matmul(
                                    pv[:, j0:j0 + jw],
                                    hc[:, k, t * P:(t + 1) * P],
                                    wqkv_sb[:, 2, k, j0:j0 + jw],
                                    start=(k == 0), stop=(k == HT - 1))
                        nc.vector.tensor_copy(vn8[:, tt, :], pv[:, :H])
                    if ci + 2 < len(cks):
                        stats1(l, ci + 2)
                lw_qkv.release()

                # prefetch W1 during attention (W2 after Wo, hides under W1 phase)
                lw_ffn = tc.alloc_tile_pool(name=f"wffn{l}", bufs=1)
                w1_sb = lw_ffn.tile([P, HT, FF], BF16, tag="w1", name="w1")
                nc.sync.dma_start(out=w1_sb, in_=w1_x[l].rearrange("c p j -> p c j"))
                lw_att = tc.alloc_tile_pool(name=f"wo{l}", bufs=1)
                wo_sb = lw_att.tile([P, HT, H], BF16, tag="wo", name="wo")
                nc.sync.dma_start(out=wo_sb, in_=wo_x[l].rearrange("c p j -> p c j"))

                # ===== attention =====
                f_cks = [(S - P, P)] if last else cks
                pend2 = {}

                def stats2(ci):
                    c0, cw = f_cks[ci]
                    mn, rstd = ln_stats(pp, gps, c0, cw, "mm", 4)
                    hc = pp.tile([P, HT, 512], BF16, tag="hc", bufs=2, name="hc2")
                    ln_apply(pp, mn, rstd, c0, cw, hc, 0)
                    pend2[ci] = hc

                q_blocks = [(S - P, P)] if last else cks
                for (q0, qw) in q_blocks:
                    ktn = (q0 + qw) // P
                    attnT = lay.tile([P, NT, 512], qk_dt, tag="attnT", name="attnT")
                    for kt in range(ktn):
                        lo = kt * P - q0
                        rl = max(lo, 0)   # skip fully-masked columns
                        scp = gps.tile([P, 512], F32, tag="mm", bufs=4,
                                       name="scp", space="PSUM")
                        if ATTN_FP8:
                            for c in range(0, HT, 2):
                                nc.tensor.matmul(
                                    scp[:, rl:qw],
                                    kt8[:, c:c + 2, kt * P:(kt + 1) * P],
                                    qt8[:, c:c + 2, q0 + rl:q0 + qw],
                                    start=(c == 0), stop=(c == HT - 2),
                                    perf_mode=DRMODE)
                        else:
                            for c in range(HT):
                                nc.tensor.matmul(
                                    scp[:, rl:qw], kt8[:, c, kt * P:(kt + 1) * P],
                                    qt8[:, c, q0 + rl:q0 + qw],
                                    start=(c == 0), stop=(c == HT - 1))
                        if lo + P <= 0:
                            nc.scalar.activation(out=attnT[:, kt, :qw],
                                                 in_=scp[:, :qw], func=AF.Exp,
                                                 bias=ln16_t[:], scale=SCALE)
                        else:
                            if lo > 0:
                                nc.vector.memset(attnT[:, kt, 0:lo], 0.0)
                            d0 = max(lo, 0)
                            d1 = min(lo + P, qw)
                            dt_ = lay.tile([P, P], BF16, tag="dtmp", bufs=1,
                                           name="dtmp")
                            nc.scalar.activation(out=dt_[:, :d1 - d0],
                                                 in_=scp[:, d0:d1], func=AF.Exp,
                                                 bias=ln16_t[:], scale=SCALE)
                            nc.vector.tensor_mul(attnT[:, kt, d0:d1],
                                                 dt_[:, :d1 - d0],
                                                 cmask[:, d0 - lo:d1 - lo])
                            if d1 < qw:
                                nc.scalar.activation(out=attnT[:, kt, d1:qw],
                                                     in_=scp[:, d1:qw], func=AF.Exp,
                                                     bias=ln16_t[:], scale=SCALE)
                    # denominator
                    dnp = gps.tile([P, 512], F32, tag="mm", bufs=4, name="dnp",
                                   space="PSUM")
                    if ATTN_FP8:
                        for kt in range(0, ktn, 2):
                            nc.tensor.matmul(dnp[:, :qw], ones_8,
                                             attnT[:, kt:kt + 2, :qw],
                                             start=(kt == 0), stop=(kt == ktn - 2),
                                             perf_mode=DRMODE)
                    else:
                        for kt in range(ktn):
                            nc.tensor.matmul(dnp[:, :qw], ones_b,
                                             attnT[:, kt, :qw],
                                             start=(kt == 0), stop=(kt == ktn - 1))
                    dnl = lay.tile([P, 512], F32, tag="dnl", bufs=1, name="dnl")
                    nc.scalar.activation(out=dnl[:, :qw], in_=dnp[:, :qw],
                                         func=AF.Ln)
                    dn = lay.tile([P, 512], BF16, tag="dn", bufs=1, name="dn")
                    nc.scalar.activation(out=dn[:, :qw], in_=dnl[:, :qw],
                                         func=AF.Exp, scale=-1.0)
                    # AV
                    ot = lay.tile([P, HT, 512], BF16, tag="ot", bufs=1, name="ot")
                    for m in range(HT):
                        avp = gps.tile([P, 512], F32, tag="mm", bufs=4,
                                       name="avp", space="PSUM")
                        if ATTN_FP8:
                            for kt in range(0, ktn, 2):
                                nc.tensor.matmul(
                                    avp[:, :qw], vn8[:, kt:kt + 2, m * P:(m + 1) * P],
                                    attnT[:, kt:kt + 2, :qw],
                                    start=(kt == 0), stop=(kt == ktn - 2),
                                    perf_mode=DRMODE)
                        else:
                            for kt in range(ktn):
                                nc.tensor.matmul(
                                    avp[:, :qw], vn8[:, kt, m * P:(m + 1) * P],
                                    attnT[:, kt, :qw],
                                    start=(kt == 0), stop=(kt == ktn - 1))
                        nc.vector.tensor_copy(ot[:, m, :qw], avp[:, :qw])
                    # Wo + deferred softmax-normalization + residual
                    for m2 in range(HT):
                        wop = gps.tile([P, 512], F32, tag="mm", bufs=4,
                                       name="wop", space="PSUM")
                        for k in range(HT):
                            nc.tensor.matmul(wop[:, :qw],
                                             wo_sb[:, k, m2 * P:(m2 + 1) * P],
                                             ot[:, k, :qw],
                                             start=(k == 0), stop=(k == HT - 1))
                        wot = lay.tile([P, 512], F32, tag="wot", bufs=2,
                                       name="wot")
                        nc.vector.tensor_mul(wot[:, :qw], wop[:, :qw], dn[:, :qw])
                        nc.vector.scalar_tensor_tensor(
                            out=xT[:, m2, q0:q0 + qw], in0=wot[:, :qw],
                            scalar=ppp[:, _BO + m2:_BO + m2 + 1],
                            in1=xT[:, m2, q0:q0 + qw], op0=OP.add, op1=OP.add)
                # LN2 stats for first two FFN chunks (xT finalized above)
                stats2(0)
                if len(f_cks) > 1:
                    stats2(1)
                lw_att.release()

                w2_sb = lw_ffn.tile([P, FT, H], BF16, tag="w2", name="w2")
                nc.sync.dma_start(out=w2_sb, in_=w2_x[l].rearrange("c p j -> p c j"))

                # ===== FFN =====
                g1 = lay.tile([P, FT, 512], BF16, tag="g1", name="g1")
                for ci, (c0, cw) in enumerate(f_cks):
                    hc = pend2.pop(ci)
                    for m in range(FT):
                        f1p = gps.tile([P, 512], F32, tag="mm", bufs=4,
                                       name="f1p", space="PSUM")
                        for k in range(HT):
                            nc.tensor.matmul(
                                f1p[:, :cw], w1_sb[:, k, m * P:(m + 1) * P],
                                hc[:, k, :cw],
                                start=(k == 0), stop=(k == HT - 1))
                        nc.scalar.activation(out=g1[:, m, :cw], in_=f1p[:, :cw],
                                             func=AF.Gelu,
                                             bias=ppp[:, _B1 + m:_B1 + m + 1])
                    for m2 in range(HT):
                        f2p = gps.tile([P, 512], F32, tag="mm", bufs=4,
                                       name="f2p", space="PSUM")
                        for k in range(FT):
                            nc.tensor.matmul(
                                f2p[:, :cw], w2_sb[:, k, m2 * P:(m2 + 1) * P],
                                g1[:, k, :cw],
                                start=(k == 0), stop=(k == FT - 1))
                        nc.vector.scalar_tensor_tensor(
                            out=xT[:, m2, c0:c0 + cw], in0=f2p[:, :cw],
                            scalar=ppp[:, _B2 + m2:_B2 + m2 + 1],
                            in1=xT[:, m2, c0:c0 + cw], op0=OP.add, op1=OP.add)
                    if ci + 2 < len(f_cks):
                        stats2(ci + 2)
                # LN1 stats of next layer for chunks 0/1 (residual final)
                if l + 1 < L:
                    stats1(l + 1, 0)
                    stats1(l + 1, 1)
                lw_ffn.release()
            gps.release()

            # ---- final LN on last column + classifier (f32 small ops) ----
            with tc.tile_pool(name="head", bufs=1) as sb, \
                 tc.tile_pool(name="headp", bufs=1, space="PSUM") as ps:
                col = S - 1
                xcol = sb.tile([P, HT, 1], F32, tag="hxcol")
                nc.vector.tensor_copy(xcol, xT[:, :, col:col + 1])
                ones_f = sb.tile([P, P], F32, tag="hones")
                nc.vector.memset(ones_f, 1.0)
                s1 = ps.tile([P, 1], F32, tag="hp", bufs=2, space="PSUM")
                for c in range(HT):
                    nc.tensor.matmul(s1, ones_f, xcol[:, c, :],
                                     start=(c == 0), stop=(c == HT - 1))
                sqc = sb.tile([P, HT, 1], F32, tag="hsq")
                nc.vector.tensor_mul(sqc, xcol, xcol)
                s2 = ps.tile([P, 1], F32, tag="hp", bufs=2, space="PSUM")
                for c in range(HT):
                    nc.tensor.matmul(s2, ones_f, sqc[:, c, :],
                                     start=(c == 0), stop=(c == HT - 1))
                mean = sb.tile([P, 1], F32, tag="hmean")
                nc.vector.tensor_scalar_mul(mean, s1, 1.0 / H)
                rstd = sb.tile([P, 1], F32, tag="hrstd")
                nc.vector.tensor_scalar_mul(rstd, s2, 1.0 / H)
                m2_ = sb.tile([P, 1], F32, tag="hm2")
                nc.vector.tensor_mul(m2_, mean, mean)
                nc.vector.tensor_sub(rstd, rstd, m2_)
                nc.scalar.activation(out=rstd, in_=rstd, func=AF.Sqrt,
                                     bias=eps_t[:])
                nc.vector.reciprocal(rstd, rstd)
                nf = sb.tile([P, HT, 1], F32, tag="hnf")
                for c in range(HT):
                    t = sb.tile([P, 1], F32, tag="ht", bufs=2)
                    nc.vector.tensor_sub(t, xcol[:, c, :], mean)
                    nc.vector.tensor_mul(t, t, rstd)
                    nc.vector.tensor_scalar(
                        out=nf[:, c, :], in0=t,
                        scalar1=fpp[:, c:c + 1], scalar2=fpp[:, 6 + c:7 + c],
                        op0=OP.mult, op1=OP.add)
                lp = ps.tile([P, 1], F32, tag="hp", bufs=2, space="PSUM")
                for c in range(HT):
                    nc.tensor.matmul(lp[0:1, :], nf[:, c, :], fpp[:, 12 + c:13 + c],
                                     start=(c == 0), stop=(c == HT - 1))
                cb = sb.tile([P, 1], F32, tag="hcb")
                nc.sync.dma_start(out=cb[0:1, :], in_=clsb_x[:])
                oo = sb.tile([P, 1], F32, tag="hoo")
                nc.vector.tensor_add(oo[0:1, :], lp[0:1, :], cb[0:1, :])
                nc.sync.dma_start(out=out_x[:], in_=oo[0:1, :])

    nc.finalize()
    return nc


def _pack_host(inputs, S, L):
    f32 = np.float32
    bf16 = ml_dtypes.bfloat16

    def npf(x):
        return np.asarray(x, dtype=f32)

    Wq, Wk, Wv = npf(inputs["Wq"]), npf(inputs["Wk"]), npf(inputs["Wv"])
    Wo, W1, W2 = npf(inputs["Wo"]), npf(inputs["W1"]), npf(inputs["W2"])
    g1 = npf(inputs["ln1_g"])[:, :, None]
    b1 = npf(inputs["ln1_b"])
    g2 = npf(inputs["ln2_g"])[:, :, None]
    b2 = npf(inputs["ln2_b"])

    Wq_e, Wk_e, Wv_e = g1 * Wq, g1 * Wk, g1 * Wv
    W1_e = g2 * W1

    bq_e = np.einsum('lh,lhj->lj', b1, Wq) + npf(inputs["bq"])
    bv_e = np.einsum('lh,lhj->lj', b1, Wv) + npf(inputs["bv"])
    bo_e = npf(inputs["bo"]) + np.einsum('lh,lhj->lj', bv_e, Wo)
    b1_e = np.einsum('lh,lhj->lj', b2, W1) + npf(inputs["b1"])
    b2_e = npf(inputs["b2"])

    wqkv = np.stack([Wq_e, Wk_e, Wv_e], axis=1).reshape(L, 3, HT, P, H).astype(bf16)
    wo_p = Wo.reshape(L, HT, P, H).astype(bf16)
    w1_p = W1_e.reshape(L, HT, P, FF).astype(bf16)
    w2_p = W2.reshape(L, FT, P, H).astype(bf16)

    def pcol(v, n):  # [L, n*128] -> [L, 128, n]
        return np.ascontiguousarray(np.transpose(v.reshape(L, n, P), (0, 2, 1)))

    ppp = np.concatenate([
        pcol(bq_e, HT), pcol(bo_e, HT), pcol(b2_e, HT), pcol(b1_e, FT),
    ], axis=2)
    assert ppp.shape == (L, P, _PPPW)

    fpp = np.concatenate([
        npf(inputs["fln_g"]).reshape(HT, P).T,
        npf(inputs["fln_b"]).reshape(HT, P).T,
        npf(inputs["cls_W"]).reshape(HT, P).T,
    ], axis=1)

    cm = (np.arange(P)[None, :] >= np.arange(P)[:, None])  # [k, q] valid q>=k

    return {
        "tok_emb": npf(inputs["tok_emb"]),
        "pos": npf(inputs["pos_emb"])[:S],
        "ident": np.eye(P, dtype=f32),
        "cmask": cm.astype(bf16),
        "wqkv": wqkv, "wo_p": wo_p, "w1_p": w1_p, "w2_p": w2_p,
        "ppp": ppp,
        "fpp": np.ascontiguousarray(fpp),
        "clsb": npf(inputs["cls_b"]).reshape(1, 1),
    }


_NC_CACHE = {}


def run_model(inputs, S=S_FULL, L=L_FULL, B=B_FULL, q_last=True, trace=False):
    mask = np.asarray(inputs["attention_mask"])
    if not np.all(mask == 1):
        raise NotImplementedError("padded attention_mask not supported")

    shared = _pack_host(inputs, S, L)
    ids = np.asarray(inputs["input_ids"]).astype(np.int32)  # [B, S]
    in_maps = []
    for b in range(B):
        m = dict(shared)
        m["ids32"] = np.ascontiguousarray(ids[b].reshape(S // P, P, 1))
        in_maps.append(m)

    key = (S, L, q_last)
    if key not in _NC_CACHE:
        _NC_CACHE[key] = build_nc(S, L, q_last)
    nc = _NC_CACHE[key]

    res = run_bass_kernel_spmd(nc, in_maps, list(range(B)), trace=trace)
    out = np.stack([res.results[b]["out"].reshape(1) for b in range(B)], axis=0)
    return out.astype(np.float32), res


def kernel(**inputs) -> np.ndarray:
    out, _ = run_model(inputs, S=S_FULL, L=L_FULL, B=B_FULL)
    return out


# revision 3
# speedup vs baseline: 1.0465x; 1.0292x over previous
"""Trainium2 Bass kernel v2: 6-layer causal transformer binary classifier.

Data-parallel over batch: B=8 rows -> 8 NeuronCores. Activations transposed
([H, S], H on partitions). Residual stream in bf16. LN gamma/beta folded into
the following weights/biases on the host (k-bias dropped: softmax-row
invariant; v-bias folded into Wo bias via softmax normalization). All big
GEMMs bf16 with full-rate back-to-back streams (LDWEIGHTS hidden); attention
scores/AV in fp8 e4m3 with DoubleRow (2x PE rate). 1/sqrt(H) folded into the
softmax exp; exp scaled by 16 into fp8 range, cancels in normalization.
"""

import numpy as np
import ml_dtypes

import concourse.bass as bass
import concourse.mybir as mybir
import concourse.tile as tile
from concourse import bacc
from concourse.bass_utils import run_bass_kernel_spmd

F32 = mybir.dt.float32
BF16 = mybir.dt.bfloat16
FP8 = mybir.dt.float8e4
I32 = mybir.dt.int32
DRMODE = mybir.MatmulPerfMode.DoubleRow

L_FULL, B_FULL, S_FULL, H, V = 6, 8, 2048, 768, 32000
FF = 4 * H
EPS = 1e-5
P = 128
HT = H // P          # 6
FT = FF // P         # 24

# ppp column layout per layer: bq'(HT), bo'(HT), b2(HT), b1'(FT)
_BQ, _BO, _B2, _B1 = 0, 6, 12, 18
_PPPW = 18 + FT      # 42

AF = mybir.ActivationFunctionType
OP = mybir.AluOpType

ATTN_FP8 = True
LN16 = float(np.log(16.0))
SCALE = 1.0 / float(np.sqrt(H))


def _chunks(total, width):
    out = []
    c = 0
    while c < total:
        w = min(width, total - c)
        out.append((c, w))
        c += w
    return out


def build_nc(S, L, q_last=True):
    NT = S // P
    nc = bacc.Bacc("TRN2")

    emb_x = nc.declare_dram_parameter("tok_emb", [V, H], F32, isOutput=False)
    pos_x = nc.declare_dram_parameter("pos", [S, H], F32, isOutput=False)
    ids_x = nc.declare_dram_parameter("ids32", [NT, P, 1], I32, isOutput=False)
    ident_x = nc.declare_dram_parameter("ident", [P, P], F32, isOutput=False)
    cmask_x = nc.declare_dram_parameter("cmask", [P, P], BF16, isOutput=False)
    wqk8_x = nc.declare_dram_parameter("wqk8", [L, 2, HT, P, H], FP8, isOutput=False)
    wv_x = nc.declare_dram_parameter("wv_p", [L, HT, P, H], BF16, isOutput=False)
    wo_x = nc.declare_dram_parameter("wo_p", [L, HT, P, H], BF16, isOutput=False)
    w1_x = nc.declare_dram_parameter("w1_p", [L, HT, P, FF], BF16, isOutput=False)
    w2_x = nc.declare_dram_parameter("w2_p", [L, FT, P, H], BF16, isOutput=False)
    ppp_x = nc.declare_dram_parameter("ppp", [L, P, _PPPW], F32, isOutput=False)
    fpp_x = nc.declare_dram_parameter("fpp", [P, 18], F32, isOutput=False)
    clsb_x = nc.declare_dram_parameter("clsb", [1, 1], F32, isOutput=False)
    out_x = nc.declare_dram_parameter("out", [1, 1], F32, isOutput=True)

    qk_dt = FP8 if ATTN_FP8 else BF16

    with tile.TileContext(nc) as tc:
        with tc.tile_pool(name="persist", bufs=1) as pp:
            cmask = pp.tile([P, P], BF16, tag="cmask")
            nc.sync.dma_start(out=cmask, in_=cmask_x[:])
            ones_b = pp.tile([P, P], BF16, tag="ones_b")
            nc.vector.memset(ones_b, 1.0)
            ones_8 = pp.tile([P, 2, P], qk_dt, tag="ones_8")
            nc.vector.memset(ones_8, 1.0)
            eps_t = pp.tile([P, 1], F32, tag="eps")
            nc.vector.memset(eps_t, EPS)
            ln16_t = pp.tile([P, 1], F32, tag="ln16")
            nc.vector.memset(ln16_t, LN16 if ATTN_FP8 else 0.0)
            fpp = pp.tile([P, 18], F32, tag="fpp")
            nc.sync.dma_start(out=fpp, in_=fpp_x[:])

            xT = pp.tile([P, HT, S], BF16, tag="xT")
            qt8 = pp.tile([P, HT, S], qk_dt, tag="qt8")
            kt8 = pp.tile([P, HT, S], qk_dt, tag="kt8")
            vn8 = pp.tile([P, NT, H], qk_dt, tag="vn8")

            # ---- embedding: gather + pos, PE-transpose into xT (bf16) ----
            with tc.tile_pool(name="emb", bufs=1) as ep, \
                 tc.tile_pool(name="embp", bufs=1, space="PSUM") as epp:
                ident = ep.tile([P, P], F32, tag="ident")
                nc.sync.dma_start(out=ident, in_=ident_x[:])
                for tt in range(NT):
                    ids_t = ep.tile([P, 1], I32, tag="ids", bufs=4)
                    nc.sync.dma_start(out=ids_t, in_=ids_x[tt])
                    xn = ep.tile([P, H], F32, tag="xn", bufs=6)
                    nc.gpsimd.indirect_dma_start(
                        out=xn[:], out_offset=None, in_=emb_x[:],
                        in_offset=bass.IndirectOffsetOnAxis(ap=ids_t[:, :1], axis=0))
                    pos_t = ep.tile([P, H], F32, tag="pos", bufs=4)
                    nc.sync.dma_start(out=pos_t, in_=pos_x[tt * P:(tt + 1) * P, :])
                    nc.vector.tensor_add(xn, xn, pos_t)
                    for c in range(HT):
                        trp = epp.tile([P, P], F32, tag="tr", bufs=4, space="PSUM")
                        nc.tensor.transpose(out=trp[:], in_=xn[:, c * P:(c + 1) * P],
                                            identity=ident[:])
                        nc.vector.tensor_copy(xT[:, c, tt * P:(tt + 1) * P], trp)

            # ---- LN stats helper (emits PE matmuls + DVE math) ----
            def ln_stats(sb, ps, c0, w, ptag, pbufs):
                """returns (mn bf16 [P,w], rstd bf16 [P,w])"""
                s1 = ps.tile([P, 512], F32, tag=ptag, bufs=pbufs, name="s1",
                             space="PSUM")
                for c in range(HT):
                    nc.tensor.matmul(s1[:, :w], ones_b, xT[:, c, c0:c0 + w],
                                     start=(c == 0), stop=(c == HT - 1))
                s2 = ps.tile([P, 512], F32, tag=ptag, bufs=pbufs, name="s2",
                             space="PSUM")
                for c in range(HT):
                    sq = sb.tile([P, 512], BF16, tag="sq", bufs=1, name="sq")
                    nc.vector.tensor_mul(sq[:, :w], xT[:, c, c0:c0 + w],
                                         xT[:, c, c0:c0 + w])
                    nc.tensor.matmul(s2[:, :w], ones_b, sq[:, :w],
                                     start=(c == 0), stop=(c == HT - 1))
                mn = sb.tile([P, 512], BF16, tag="mn", bufs=2, name="mn")
                nc.vector.tensor_scalar_mul(mn[:, :w], s1[:, :w], 1.0 / H)
                m2 = sb.tile([P, 512], BF16, tag="lnt", bufs=1, name="m2")
                nc.vector.tensor_mul(m2[:, :w], mn[:, :w], mn[:, :w])
                rstd = sb.tile([P, 512], BF16, tag="rstd", bufs=2, name="rstd")
                rs32 = sb.tile([P, 512], BF16, tag="sq", bufs=1, name="rs32")
                nc.vector.scalar_tensor_tensor(
                    out=rs32[:, :w], in0=s2[:, :w], scalar=1.0 / H,
                    in1=m2[:, :w], op0=OP.mult, op1=OP.subtract)
                nc.scalar.activation(out=rstd[:, :w], in_=rs32[:, :w],
                                     func=AF.Abs_reciprocal_sqrt, bias=eps_t[:])
                return mn, rstd

            def ln_apply(sb, mn, rstd, c0, w, out_tile, out_off):
                for c in range(HT):
                    t = sb.tile([P, 512], BF16, tag="lnt", bufs=1, name="lnt")
                    nc.vector.tensor_sub(t[:, :w], xT[:, c, c0:c0 + w], mn[:, :w])
                    nc.vector.tensor_mul(out_tile[:, c, out_off:out_off + w],
                                         t[:, :w], rstd[:, :w])

            # ---- transformer layers ----
            gps = tc.alloc_tile_pool(name="gps", bufs=1, space="PSUM")
            cks = _chunks(S, 512)
            pend1 = {}

            def stats1(l_, ci):
                c0, cw = cks[ci]
                mn, rstd = ln_stats(pp, gps, c0, cw, "mm", 4)
                hc = pp.tile([P, HT, 512], BF16, tag="hc", bufs=2, name="hc1")
                ln_apply(pp, mn, rstd, c0, cw, hc, 0)
                h8 = pp.tile([P, HT, 512], FP8, tag="h8", bufs=2, name="h8")
                for c in range(HT):
                    nc.vector.tensor_copy(h8[:, c, :cw], hc[:, c, :cw])
                pend1[(l_, ci)] = (hc, h8)

            for l in range(L):
                last = q_last and (l == L - 1)

                lw_qkv = tc.alloc_tile_pool(name=f"wqkv{l}", bufs=1)
                wqk8_sb = lw_qkv.tile([P, 2, HT, H], FP8, tag="wqk8", name="wqk8")
                nc.sync.dma_start(
                    out=wqk8_sb, in_=wqk8_x[l].rearrange("i c p j -> p i c j"))
                wv_sb = lw_qkv.tile([P, HT, H], BF16, tag="wv", name="wv")
                nc.sync.dma_start(out=wv_sb, in_=wv_x[l].rearrange("c p j -> p c j"))
                lay = pp
                ppp = pp.tile([P, _PPPW], F32, tag="ppp", bufs=2, name=f"ppp{l}")
                nc.sync.dma_start(out=ppp, in_=ppp_x[l])

                # ===== QKV phase (stats pipelined two chunks ahead) =====
                for ci in (0, 1):
                    if ci < len(cks) and (l, ci) not in pend1:
                        stats1(l, ci)
                for ci, (c0, cw) in enumerate(cks):
                    hc, h8 = pend1.pop((l, ci))
                    # Q (skip unless needed): fp8 DoubleRow
                    if (not last) or (c0 + cw == S):
                        qo, qcw = (384, P) if last else (0, cw)
                        for m in range(HT):
                            pj = gps.tile([P, 512], F32, tag="mm", bufs=4,
                                          name="pjq", space="PSUM")
                            for k in range(0, HT, 2):
                                nc.tensor.matmul(
                                    pj[:, :qcw],
                                    wqk8_sb[:, 0, k:k + 2, m * P:(m + 1) * P],
                                    h8[:, k:k + 2, qo:qo + qcw],
                                    start=(k == 0), stop=(k == HT - 2),
                                    perf_mode=DRMODE)
                            nc.vector.tensor_scalar(
                                out=qt8[:, m, c0 + qo:c0 + qo + qcw],
                                in0=pj[:, :qcw], scalar1=1.0 / 64.0,
                                scalar2=ppp[:, _BQ + m:_BQ + m + 1],
                                op0=OP.mult, op1=OP.add)
                    # K: fp8 DoubleRow, no bias
                    for m in range(HT):
                        pj = gps.tile([P, 512], F32, tag="mm", bufs=4,
                                      name="pjk", space="PSUM")
                        for k in range(0, HT, 2):
                            nc.tensor.matmul(
                                pj[:, :cw],
                                wqk8_sb[:, 1, k:k + 2, m * P:(m + 1) * P],
                                h8[:, k:k + 2, :cw],
                                start=(k == 0), stop=(k == HT - 2),
                                perf_mode=DRMODE)
                        nc.vector.tensor_scalar_mul(kt8[:, m, c0:c0 + cw],
                                                    pj[:, :cw], 1.0 / 64.0)
                    # V: stationary = hc token-tile, moving = weights
                    for t in range(cw // P):
                        tt = (c0 // P) + t
                        pv = gps.tile([P, 1024], F32, tag="vp", bufs=2,
                                      name="pv", space="PSUM")
                        for (j0, jw) in _chunks(H, 512):
                            for k in range(HT):
                                nc.tensor.matmul(
                                    pv[:, j0:j0 + jw],
                                    hc[:, k, t * P:(t + 1) * P],
                                    wv_sb[:, k, j0:j0 + jw],
                                    start=(k == 0), stop=(k == HT - 1))
                        nc.vector.tensor_copy(vn8[:, tt, :], pv[:, :H])
                    if ci + 2 < len(cks):
                        stats1(l, ci + 2)
                lw_qkv.release()

                # prefetch W1 during attention (W2 after Wo, hides under W1 phase)
                lw_ffn = tc.alloc_tile_pool(name=f"wffn{l}", bufs=1)
                w1_sb = lw_ffn.tile([P, HT, FF], BF16, tag="w1", name="w1")
                nc.sync.dma_start(out=w1_sb, in_=w1_x[l].rearrange("c p j -> p c j"))
                lw_att = tc.alloc_tile_pool(name=f"wo{l}", bufs=1)
                wo_sb = lw_att.tile([P, HT, H], BF16, tag="wo", name="wo")
                nc.sync.dma_start(out=wo_sb, in_=wo_x[l].rearrange("c p j -> p c j"))

                # ===== attention =====
                f_cks = [(S - P, P)] if last else _chunks(S, 384)
                pend2 = {}

                def stats2(ci):
                    c0, cw = f_cks[ci]
                    mn, rstd = ln_stats(pp, gps, c0, cw, "mm", 4)
                    hc = pp.tile([P, HT, 512], BF16, tag="hc", bufs=2, name="hc2")
                    ln_apply(pp, mn, rstd, c0, cw, hc, 0)
                    pend2[ci] = hc

                q_blocks = [(S - P, P)] if last else cks
                for (q0, qw) in q_blocks:
                    ktn = (q0 + qw) // P
                    attnT = lay.tile([P, NT, 512], qk_dt, tag="attnT", name="attnT")
                    for kt in range(ktn):
                        lo = kt * P - q0
                        rl = max(lo, 0)   # skip fully-masked columns
                        scp = gps.tile([P, 512], F32, tag="mm", bufs=4,
                                       name="scp", space="PSUM")
                        if ATTN_FP8:
                            for c in range(0, HT, 2):
                                nc.tensor.matmul(
                                    scp[:, rl:qw],
                                    kt8[:, c:c + 2, kt * P:(kt + 1) * P],
                                    qt8[:, c:c + 2, q0 + rl:q0 + qw],
                                    start=(c == 0), stop=(c == HT - 2),
                                    perf_mode=DRMODE)
                        else:
                            for c in range(HT):
                                nc.tensor.matmul(
                                    scp[:, rl:qw], kt8[:, c, kt * P:(kt + 1) * P],
                                    qt8[:, c, q0 + rl:q0 + qw],
                                    start=(c == 0), stop=(c == HT - 1))
                        if lo + P <= 0:
                            nc.scalar.activation(out=attnT[:, kt, :qw],
                                                 in_=scp[:, :qw], func=AF.Exp,
                                                 bias=ln16_t[:], scale=SCALE)
                        else:
                            if lo > 0:
                                nc.vector.memset(attnT[:, kt, 0:lo], 0.0)
                            d0 = max(lo, 0)
                            d1 = min(lo + P, qw)
                            dt_ = lay.tile([P, P], BF16, tag="dtmp", bufs=1,
                                           name="dtmp")
                            nc.scalar.activation(out=dt_[:, :d1 - d0],
                                                 in_=scp[:, d0:d1], func=AF.Exp,
                                                 bias=ln16_t[:], scale=SCALE)
                            nc.vector.tensor_mul(attnT[:, kt, d0:d1],
                                                 dt_[:, :d1 - d0],
                                                 cmask[:, d0 - lo:d1 - lo])
                            if d1 < qw:
                                nc.scalar.activation(out=attnT[:, kt, d1:qw],
                                                     in_=scp[:, d1:qw], func=AF.Exp,
                                                     bias=ln16_t[:], scale=SCALE)
                    # denominator
                    dnp = gps.tile([P, 512], F32, tag="mm", bufs=4, name="dnp",
                                   space="PSUM")
                    if ATTN_FP8:
                        for kt in range(0, ktn, 2):
                            nc.tensor.matmul(dnp[:, :qw], ones_8,
                                             attnT[:, kt:kt + 2, :qw],
                                             start=(kt == 0), stop=(kt == ktn - 2),
                                             perf_mode=DRMODE)
                    else:
                        for kt in range(ktn):
                            nc.tensor.matmul(dnp[:, :qw], ones_b,
                                             attnT[:, kt, :qw],
                                             start=(kt == 0), stop=(kt == ktn - 1))
                    dnl = lay.tile([P, 512], F32, tag="wot", bufs=2, name="dnl")
                    nc.scalar.activation(out=dnl[:, :qw], in_=dnp[:, :qw],
                                         func=AF.Ln)
                    dn = lay.tile([P, 512], BF16, tag="dn", bufs=1, name="dn")
                    nc.scalar.activation(out=dn[:, :qw], in_=dnl[:, :qw],
                                         func=AF.Exp, scale=-1.0)
                    # AV
                    ot = lay.tile([P, HT, 512], BF16, tag="ot", bufs=1, name="ot")
                    for m in range(HT):
                        avp = gps.tile([P, 512], F32, tag="mm", bufs=4,
                                       name="avp", space="PSUM")
                        if ATTN_FP8:
                            for kt in range(0, ktn, 2):
                                nc.tensor.matmul(
                                    avp[:, :qw], vn8[:, kt:kt + 2, m * P:(m + 1) * P],
                                    attnT[:, kt:kt + 2, :qw],
                                    start=(kt == 0), stop=(kt == ktn - 2),
                                    perf_mode=DRMODE)
                        else:
                            for kt in range(ktn):
                                nc.tensor.matmul(
                                    avp[:, :qw], vn8[:, kt, m * P:(m + 1) * P],
                                    attnT[:, kt, :qw],
                                    start=(kt == 0), stop=(kt == ktn - 1))
                        nc.vector.tensor_copy(ot[:, m, :qw], avp[:, :qw])
                    # Wo + deferred softmax-normalization + residual
                    for m2 in range(HT):
                        wop = gps.tile([P, 512], F32, tag="mm", bufs=4,
                                       name="wop", space="PSUM")
                        for k in range(HT):
                            nc.tensor.matmul(wop[:, :qw],
                                             wo_sb[:, k, m2 * P:(m2 + 1) * P],
                                             ot[:, k, :qw],
                                             start=(k == 0), stop=(k == HT - 1))
                        wot = lay.tile([P, 512], F32, tag="wot", bufs=2,
                                       name="wot")
                        nc.vector.tensor_mul(wot[:, :qw], wop[:, :qw], dn[:, :qw])
                        nc.vector.scalar_tensor_tensor(
                            out=xT[:, m2, q0:q0 + qw], in0=wot[:, :qw],
                            scalar=ppp[:, _BO + m2:_BO + m2 + 1],
                            in1=xT[:, m2, q0:q0 + qw], op0=OP.add, op1=OP.add)
                # LN2 stats for first two FFN chunks (xT finalized above)
                stats2(0)
                if len(f_cks) > 1:
                    stats2(1)
                lw_att.release()

                w2_sb = lw_ffn.tile([P, FT, H], BF16, tag="w2", name="w2")
                nc.sync.dma_start(out=w2_sb, in_=w2_x[l].rearrange("c p j -> p c j"))

                # ===== FFN =====
                g1 = lay.tile([P, FT, 384], BF16, tag="g1", name="g1")
                for ci, (c0, cw) in enumerate(f_cks):
                    hc = pend2.pop(ci)
                    for m in range(FT):
                        f1p = gps.tile([P, 512], F32, tag="mm", bufs=4,
                                       name="f1p", space="PSUM")
                        for k in range(HT):
                            nc.tensor.matmul(
                                f1p[:, :cw], w1_sb[:, k, m * P:(m + 1) * P],
                                hc[:, k, :cw],
                                start=(k == 0), stop=(k == HT - 1))
                        nc.scalar.activation(out=g1[:, m, :cw], in_=f1p[:, :cw],
                                             func=AF.Gelu,
                                             bias=ppp[:, _B1 + m:_B1 + m + 1])
                    for m2 in range(HT):
                        f2p = gps.tile([P, 512], F32, tag="mm", bufs=4,
                                       name="f2p", space="PSUM")
                        for k in range(FT):
                            nc.tensor.matmul(
                                f2p[:, :cw], w2_sb[:, k, m2 * P:(m2 + 1) * P],
                                g1[:, k, :cw],
                                start=(k == 0), stop=(k == FT - 1))
                        nc.vector.scalar_tensor_tensor(
                            out=xT[:, m2, c0:c0 + cw], in0=f2p[:, :cw],
                            scalar=ppp[:, _B2 + m2:_B2 + m2 + 1],
                            in1=xT[:, m2, c0:c0 + cw], op0=OP.add, op1=OP.add)
                    if ci + 2 < len(f_cks):
                        stats2(ci + 2)
                # LN1 stats of next layer for chunks 0/1 (residual final)
                if l + 1 < L:
                    stats1(l + 1, 0)
                    stats1(l + 1, 1)
                lw_ffn.release()
            gps.release()

            # ---- final LN on last column + classifier (f32 small ops) ----
            with tc.tile_pool(name="head", bufs=1) as sb, \
                 tc.tile_pool(name="headp", bufs=1, space="PSUM") as ps:
                col = S - 1
                xcol = sb.tile([P, HT, 1], F32, tag="hxcol")
                nc.vector.tensor_copy(xcol, xT[:, :, col:col + 1])
                ones_f = sb.tile([P, P], F32, tag="hones")
                nc.vector.memset(ones_f, 1.0)
                s1 = ps.tile([P, 1], F32, tag="hp", bufs=2, space="PSUM")
                for c in range(HT):
                    nc.tensor.matmul(s1, ones_f, xcol[:, c, :],
                                     start=(c == 0), stop=(c == HT - 1))
                sqc = sb.tile([P, HT, 1], F32, tag="hsq")
                nc.vector.tensor_mul(sqc, xcol, xcol)
                s2 = ps.tile([P, 1], F32, tag="hp", bufs=2, space="PSUM")
                for c in range(HT):
                    nc.tensor.matmul(s2, ones_f, sqc[:, c, :],
                                     start=(c == 0), stop=(c == HT - 1))
                mean = sb.tile([P, 1], F32, tag="hmean")
                nc.vector.tensor_scalar_mul(mean, s1, 1.0 / H)
                rstd = sb.tile([P, 1], F32, tag="hrstd")
                nc.vector.tensor_scalar_mul(rstd, s2, 1.0 / H)
                m2_ = sb.tile([P, 1], F32, tag="hm2")
                nc.vector.tensor_mul(m2_, mean, mean)
                nc.vector.tensor_sub(rstd, rstd, m2_)
                nc.scalar.activation(out=rstd, in_=rstd, func=AF.Sqrt,
                                     bias=eps_t[:])
                nc.vector.reciprocal(rstd, rstd)
                nf = sb.tile([P, HT, 1], F32, tag="hnf")
                for c in range(HT):
                    t = sb.tile([P, 1], F32, tag="ht", bufs=2)
                    nc.vector.tensor_sub(t, xcol[:, c, :], mean)
                    nc.vector.tensor_mul(t, t, rstd)
                    nc.vector.tensor_scalar(
                        out=nf[:, c, :], in0=t,
                        scalar1=fpp[:, c:c + 1], scalar2=fpp[:, 6 + c:7 + c],
                        op0=OP.mult, op1=OP.add)
                lp = ps.tile([P, 1], F32, tag="hp", bufs=2, space="PSUM")
                for c in range(HT):
                    nc.tensor.matmul(lp[0:1, :], nf[:, c, :], fpp[:, 12 + c:13 + c],
                                     start=(c == 0), stop=(c == HT - 1))
                cb = sb.tile([P, 1], F32, tag="hcb")
                nc.sync.dma_start(out=cb[0:1, :], in_=clsb_x[:])
                oo = sb.tile([P, 1], F32, tag="hoo")
                nc.vector.tensor_add(oo[0:1, :], lp[0:1, :], cb[0:1, :])
                nc.sync.dma_start(out=out_x[:], in_=oo[0:1, :])

    nc.finalize()
    return nc


def _pack_host(inputs, S, L):
    f32 = np.float32
    bf16 = ml_dtypes.bfloat16

    def npf(x):
        return np.asarray(x, dtype=f32)

    Wq, Wk, Wv = npf(inputs["Wq"]), npf(inputs["Wk"]), npf(inputs["Wv"])
    Wo, W1, W2 = npf(inputs["Wo"]), npf(inputs["W1"]), npf(inputs["W2"])
    g1 = npf(inputs["ln1_g"])[:, :, None]
    b1 = npf(inputs["ln1_b"])
    g2 = npf(inputs["ln2_g"])[:, :, None]
    b2 = npf(inputs["ln2_b"])

    Wq_e, Wk_e, Wv_e = g1 * Wq, g1 * Wk, g1 * Wv
    W1_e = g2 * W1

    bq_e = np.einsum('lh,lhj->lj', b1, Wq) + npf(inputs["bq"])
    bv_e = np.einsum('lh,lhj->lj', b1, Wv) + npf(inputs["bv"])
    bo_e = npf(inputs["bo"]) + np.einsum('lh,lhj->lj', bv_e, Wo)
    b1_e = np.einsum('lh,lhj->lj', b2, W1) + npf(inputs["b1"])
    b2_e = npf(inputs["b2"])

    fp8 = ml_dtypes.float8_e4m3fn
    wqk8 = np.clip(np.stack([Wq_e, Wk_e], axis=1) * 64.0, -240, 240)
    wqk8 = wqk8.reshape(L, 2, HT, P, H).astype(fp8)
    wv_p = Wv_e.reshape(L, HT, P, H).astype(bf16)
    wo_p = Wo.reshape(L, HT, P, H).astype(bf16)
    w1_p = W1_e.reshape(L, HT, P, FF).astype(bf16)
    w2_p = W2.reshape(L, FT, P, H).astype(bf16)

    def pcol(v, n):  # [L, n*128] -> [L, 128, n]
        return np.ascontiguousarray(np.transpose(v.reshape(L, n, P), (0, 2, 1)))

    ppp = np.concatenate([
        pcol(bq_e, HT), pcol(bo_e, HT), pcol(b2_e, HT), pcol(b1_e, FT),
    ], axis=2)
    assert ppp.shape == (L, P, _PPPW)

    fpp = np.concatenate([
        npf(inputs["fln_g"]).reshape(HT, P).T,
        npf(inputs["fln_b"]).reshape(HT, P).T,
        npf(inputs["cls_W"]).reshape(HT, P).T,
    ], axis=1)

    cm = (np.arange(P)[None, :] >= np.arange(P)[:, None])  # [k, q] valid q>=k

    return {
        "tok_emb": npf(inputs["tok_emb"]),
        "pos": npf(inputs["pos_emb"])[:S],
        "ident": np.eye(P, dtype=f32),
        "cmask": cm.astype(bf16),
        "wqk8": wqk8, "wv_p": wv_p, "wo_p": wo_p, "w1_p": w1_p, "w2_p": w2_p,
        "ppp": ppp,
        "fpp": np.ascontiguousarray(fpp),
        "clsb": npf(inputs["cls_b"]).reshape(1, 1),
    }


_NC_CACHE = {}


def run_model(inputs, S=S_FULL, L=L_FULL, B=B_FULL, q_last=True, trace=False):
    mask = np.asarray(inputs["attention_mask"])
    if not np.all(mask == 1):
        raise NotImplementedError("padded attention_mask not supported")

    shared = _pack_host(inputs, S, L)
    ids = np.asarray(inputs["input_ids"]).astype(np.int32)  # [B, S]
    in_maps = []
    for b in range(B):
        m = dict(shared)
        m["ids32"] = np.ascontiguousarray(ids[b].reshape(S // P, P, 1))
        in_maps.append(m)

    key = (S, L, q_last)
    if key not in _NC_CACHE:
        _NC_CACHE[key] = build_nc(S, L, q_last)
    nc = _NC_CACHE[key]

    res = run_bass_kernel_spmd(nc, in_maps, list(range(B)), trace=trace)
    out = np.stack([res.results[b]["out"].reshape(1) for b in range(B)], axis=0)
    return out.astype(np.float32), res


def kernel(**inputs) -> np.ndarray:
    out, _ = run_model(inputs, S=S_FULL, L=L_FULL, B=B_FULL)
    return out


# revision 4
# speedup vs baseline: 1.0478x; 1.0013x over previous
"""Trainium2 Bass kernel v2: 6-layer causal transformer binary classifier.

Data-parallel over batch: B=8 rows -> 8 NeuronCores. Activations transposed
([H, S], H on partitions). Residual stream in bf16. LN gamma/beta folded into
the following weights/biases on the host (k-bias dropped: softmax-row
invariant; v-bias folded into Wo bias via softmax normalization). All big
GEMMs bf16 with full-rate back-to-back streams (LDWEIGHTS hidden); attention
scores/AV in fp8 e4m3 with DoubleRow (2x PE rate). 1/sqrt(H) folded into the
softmax exp; exp scaled by 16 into fp8 range, cancels in normalization.
"""

import numpy as np
import ml_dtypes

import concourse.bass as bass
import concourse.mybir as mybir
import concourse.tile as tile
from concourse import bacc
from concourse.bass_utils import run_bass_kernel_spmd

F32 = mybir.dt.float32
BF16 = mybir.dt.bfloat16
FP8 = mybir.dt.float8e4
I32 = mybir.dt.int32
DRMODE = mybir.MatmulPerfMode.DoubleRow

L_FULL, B_FULL, S_FULL, H, V = 6, 8, 2048, 768, 32000
FF = 4 * H
EPS = 1e-5
P = 128
HT = H // P          # 6
FT = FF // P         # 24

# ppp column layout per layer: bq'(HT), bo'(HT), b2(HT), b1'(FT)
_BQ, _BO, _B2, _B1 = 0, 6, 12, 18
_PPPW = 18 + FT      # 42

AF = mybir.ActivationFunctionType
OP = mybir.AluOpType

ATTN_FP8 = True
LN16 = float(np.log(16.0))
SCALE = 1.0 / float(np.sqrt(H))


def _chunks(total, width):
    out = []
    c = 0
    while c < total:
        w = min(width, total - c)
        out.append((c, w))
        c += w
    return out


def build_nc(S, L, q_last=True):
    NT = S // P
    nc = bacc.Bacc("TRN2")

    emb_x = nc.declare_dram_parameter("tok_emb", [V, H], F32, isOutput=False)
    pos_x = nc.declare_dram_parameter("pos", [S, H], F32, isOutput=False)
    ids_x = nc.declare_dram_parameter("ids32", [NT, P, 1], I32, isOutput=False)
    ident_x = nc.declare_dram_parameter("ident", [P, P], F32, isOutput=False)
    cmask_x = nc.declare_dram_parameter("cmask", [P, P], BF16, isOutput=False)
    wqk8_x = nc.declare_dram_parameter("wqk8", [L, 2, HT, P, H], FP8, isOutput=False)
    wv_x = nc.declare_dram_parameter("wv_p", [L, HT, P, H], BF16, isOutput=False)
    wo_x = nc.declare_dram_parameter("wo_p", [L, HT, P, H], BF16, isOutput=False)
    w1_x = nc.declare_dram_parameter("w1_p", [L, HT, P, FF], BF16, isOutput=False)
    w2_x = nc.declare_dram_parameter("w2_p", [L, FT, P, H], BF16, isOutput=False)
    ppp_x = nc.declare_dram_parameter("ppp", [L, P, _PPPW], F32, isOutput=False)
    fpp_x = nc.declare_dram_parameter("fpp", [P, 18], F32, isOutput=False)
    clsb_x = nc.declare_dram_parameter("clsb", [1, 1], F32, isOutput=False)
    out_x = nc.declare_dram_parameter("out", [1, 1], F32, isOutput=True)

    qk_dt = FP8 if ATTN_FP8 else BF16

    with tile.TileContext(nc) as tc:
        with tc.tile_pool(name="persist", bufs=1) as pp:
            cmask = pp.tile([P, P], BF16, tag="cmask")
            nc.sync.dma_start(out=cmask, in_=cmask_x[:])
            ones_b = pp.tile([P, P], BF16, tag="ones_b")
            nc.vector.memset(ones_b, 1.0)
            ones_8 = pp.tile([P, 2, P], qk_dt, tag="ones_8")
            nc.vector.memset(ones_8, 1.0)
            eps_t = pp.tile([P, 1], F32, tag="eps")
            nc.vector.memset(eps_t, EPS)
            ln16_t = pp.tile([P, 1], F32, tag="ln16")
            nc.vector.memset(ln16_t, LN16 if ATTN_FP8 else 0.0)
            fpp = pp.tile([P, 18], F32, tag="fpp")
            nc.sync.dma_start(out=fpp, in_=fpp_x[:])

            xT = pp.tile([P, HT, S], BF16, tag="xT")
            qt8 = pp.tile([P, HT, S], qk_dt, tag="qt8")
            kt8 = pp.tile([P, HT, S], qk_dt, tag="kt8")
            vn8 = pp.tile([P, NT, H], qk_dt, tag="vn8")

            # ---- embedding: gather + pos, PE-transpose into xT (bf16) ----
            with tc.tile_pool(name="emb", bufs=1) as ep, \
                 tc.tile_pool(name="embp", bufs=1, space="PSUM") as epp:
                ident = ep.tile([P, P], F32, tag="ident")
                nc.sync.dma_start(out=ident, in_=ident_x[:])
                for tt in range(NT):
                    ids_t = ep.tile([P, 1], I32, tag="ids", bufs=4)
                    nc.sync.dma_start(out=ids_t, in_=ids_x[tt])
                    xn = ep.tile([P, H], F32, tag="xn", bufs=6)
                    nc.gpsimd.indirect_dma_start(
                        out=xn[:], out_offset=None, in_=emb_x[:],
                        in_offset=bass.IndirectOffsetOnAxis(ap=ids_t[:, :1], axis=0))
                    pos_t = ep.tile([P, H], F32, tag="pos", bufs=4)
                    nc.sync.dma_start(out=pos_t, in_=pos_x[tt * P:(tt + 1) * P, :])
                    nc.vector.tensor_add(xn, xn, pos_t)
                    for c in range(HT):
                        trp = epp.tile([P, P], F32, tag="tr", bufs=4, space="PSUM")
                        nc.tensor.transpose(out=trp[:], in_=xn[:, c * P:(c + 1) * P],
                                            identity=ident[:])
                        nc.vector.tensor_copy(xT[:, c, tt * P:(tt + 1) * P], trp)

            # ---- LN stats helper (emits PE matmuls + DVE math) ----
            def ln_stats(sb, ps, c0, w, ptag, pbufs):
                """returns (mn bf16 [P,w], rstd bf16 [P,w])"""
                s1 = ps.tile([P, 512], F32, tag=ptag, bufs=pbufs, name="s1",
                             space="PSUM")
                for c in range(HT):
                    nc.tensor.matmul(s1[:, :w], ones_b, xT[:, c, c0:c0 + w],
                                     start=(c == 0), stop=(c == HT - 1))
                s2 = ps.tile([P, 512], F32, tag=ptag, bufs=pbufs, name="s2",
                             space="PSUM")
                for c in range(HT):
                    sq = sb.tile([P, 512], BF16, tag="sq", bufs=1, name="sq")
                    nc.vector.tensor_mul(sq[:, :w], xT[:, c, c0:c0 + w],
                                         xT[:, c, c0:c0 + w])
                    nc.tensor.matmul(s2[:, :w], ones_b, sq[:, :w],
                                     start=(c == 0), stop=(c == HT - 1))
                mn = sb.tile([P, 512], BF16, tag="mn", bufs=2, name="mn")
                nc.vector.tensor_scalar_mul(mn[:, :w], s1[:, :w], 1.0 / H)
                m2 = sb.tile([P, 512], BF16, tag="lnt", bufs=1, name="m2")
                nc.vector.tensor_mul(m2[:, :w], mn[:, :w], mn[:, :w])
                rstd = sb.tile([P, 512], BF16, tag="rstd", bufs=2, name="rstd")
                rs32 = sb.tile([P, 512], BF16, tag="sq", bufs=1, name="rs32")
                nc.vector.scalar_tensor_tensor(
                    out=rs32[:, :w], in0=s2[:, :w], scalar=1.0 / H,
                    in1=m2[:, :w], op0=OP.mult, op1=OP.subtract)
                nc.scalar.activation(out=rstd[:, :w], in_=rs32[:, :w],
                                     func=AF.Abs_reciprocal_sqrt, bias=eps_t[:])
                return mn, rstd

            def ln_apply(sb, mn, rstd, c0, w, out_tile, out_off):
                for c in range(HT):
                    t = sb.tile([P, 512], BF16, tag="lnt", bufs=1, name="lnt")
                    nc.vector.tensor_sub(t[:, :w], xT[:, c, c0:c0 + w], mn[:, :w])
                    nc.vector.tensor_mul(out_tile[:, c, out_off:out_off + w],
                                         t[:, :w], rstd[:, :w])

            # ---- transformer layers ----
            gps = tc.alloc_tile_pool(name="gps", bufs=1, space="PSUM")
            cks = _chunks(S, 512)
            pend1 = {}

            def stats1(l_, ci):
                c0, cw = cks[ci]
                mn, rstd = ln_stats(pp, gps, c0, cw, "mm", 4)
                hc = pp.tile([P, HT, 512], BF16, tag="hc", bufs=2, name="hc1")
                ln_apply(pp, mn, rstd, c0, cw, hc, 0)
                h8 = pp.tile([P, HT, 512], FP8, tag="h8", bufs=2, name="h8")
                for c in range(HT):
                    nc.scalar.activation(out=h8[:, c, :cw], in_=hc[:, c, :cw],
                                         func=AF.Copy)
                pend1[(l_, ci)] = (hc, h8)

            for l in range(L):
                last = q_last and (l == L - 1)

                lw_qkv = tc.alloc_tile_pool(name=f"wqkv{l}", bufs=1)
                wqk8_sb = lw_qkv.tile([P, 2, HT, H], FP8, tag="wqk8", name="wqk8")
                nc.sync.dma_start(
                    out=wqk8_sb, in_=wqk8_x[l].rearrange("i c p j -> p i c j"))
                wv_sb = lw_qkv.tile([P, HT, H], BF16, tag="wv", name="wv")
                nc.sync.dma_start(out=wv_sb, in_=wv_x[l].rearrange("c p j -> p c j"))
                lay = pp
                ppp = pp.tile([P, _PPPW], F32, tag="ppp", bufs=2, name=f"ppp{l}")
                nc.sync.dma_start(out=ppp, in_=ppp_x[l])

                # ===== QKV phase (stats pipelined two chunks ahead) =====
                for ci in (0, 1):
                    if ci < len(cks) and (l, ci) not in pend1:
                        stats1(l, ci)
                for ci, (c0, cw) in enumerate(cks):
                    hc, h8 = pend1.pop((l, ci))
                    # Q (skip unless needed): fp8 DoubleRow
                    if (not last) or (c0 + cw == S):
                        qo, qcw = (384, P) if last else (0, cw)
                        for m in range(HT):
                            pj = gps.tile([P, 512], F32, tag="mm", bufs=4,
                                          name="pjq", space="PSUM")
                            for k in range(0, HT, 2):
                                nc.tensor.matmul(
                                    pj[:, :qcw],
                                    wqk8_sb[:, 0, k:k + 2, m * P:(m + 1) * P],
                                    h8[:, k:k + 2, qo:qo + qcw],
                                    start=(k == 0), stop=(k == HT - 2),
                                    perf_mode=DRMODE)
                            nc.vector.tensor_scalar(
                                out=qt8[:, m, c0 + qo:c0 + qo + qcw],
                                in0=pj[:, :qcw], scalar1=1.0 / 64.0,
                                scalar2=ppp[:, _BQ + m:_BQ + m + 1],
                                op0=OP.mult, op1=OP.add)
                    # K: fp8 DoubleRow, no bias
                    for m in range(HT):
                        pj = gps.tile([P, 512], F32, tag="mm", bufs=4,
                                      name="pjk", space="PSUM")
                        for k in range(0, HT, 2):
                            nc.tensor.matmul(
                                pj[:, :cw],
                                wqk8_sb[:, 1, k:k + 2, m * P:(m + 1) * P],
                                h8[:, k:k + 2, :cw],
                                start=(k == 0), stop=(k == HT - 2),
                                perf_mode=DRMODE)
                        nc.scalar.activation(out=kt8[:, m, c0:c0 + cw],
                                             in_=pj[:, :cw], func=AF.Copy,
                                             scale=1.0 / 64.0)
                    # V: stationary = hc token-tile, moving = weights
                    for t in range(cw // P):
                        tt = (c0 // P) + t
                        pv = gps.tile([P, 1024], F32, tag="vp", bufs=2,
                                      name="pv", space="PSUM")
                        for (j0, jw) in _chunks(H, 512):
                            for k in range(HT):
                                nc.tensor.matmul(
                                    pv[:, j0:j0 + jw],
                                    hc[:, k, t * P:(t + 1) * P],
                                    wv_sb[:, k, j0:j0 + jw],
                                    start=(k == 0), stop=(k == HT - 1))
                        nc.scalar.activation(out=vn8[:, tt, :], in_=pv[:, :H],
                                             func=AF.Copy)
                    if ci + 2 < len(cks):
                        stats1(l, ci + 2)
                lw_qkv.release()

                # prefetch W1 during attention (W2 after Wo, hides under W1 phase)
                lw_ffn = tc.alloc_tile_pool(name=f"wffn{l}", bufs=1)
                w1_sb = lw_ffn.tile([P, HT, FF], BF16, tag="w1", name="w1")
                nc.sync.dma_start(out=w1_sb, in_=w1_x[l].rearrange("c p j -> p c j"))
                lw_att = tc.alloc_tile_pool(name=f"wo{l}", bufs=1)
                wo_sb = lw_att.tile([P, HT, H], BF16, tag="wo", name="wo")
                nc.sync.dma_start(out=wo_sb, in_=wo_x[l].rearrange("c p j -> p c j"))

                # ===== attention =====
                f_cks = [(S - P, P)] if last else _chunks(S, 384)
                pend2 = {}

                def stats2(ci):
                    c0, cw = f_cks[ci]
                    mn, rstd = ln_stats(pp, gps, c0, cw, "mm", 4)
                    hc = pp.tile([P, HT, 512], BF16, tag="hc", bufs=2, name="hc2")
                    ln_apply(pp, mn, rstd, c0, cw, hc, 0)
                    pend2[ci] = hc

                q_blocks = [(S - P, P)] if last else cks
                for iqb, (q0, qw) in enumerate(q_blocks):
                    ktn = (q0 + qw) // P
                    attnT = lay.tile([P, NT, 512], qk_dt, tag="attnT", name="attnT")
                    for kt in range(ktn):
                        lo = kt * P - q0
                        rl = max(lo, 0)   # skip fully-masked columns
                        scp = gps.tile([P, 512], F32, tag="mm", bufs=4,
                                       name="scp", space="PSUM")
                        if ATTN_FP8:
                            for c in range(0, HT, 2):
                                nc.tensor.matmul(
                                    scp[:, rl:qw],
                                    kt8[:, c:c + 2, kt * P:(kt + 1) * P],
                                    qt8[:, c:c + 2, q0 + rl:q0 + qw],
                                    start=(c == 0), stop=(c == HT - 2),
                                    perf_mode=DRMODE)
                        else:
                            for c in range(HT):
                                nc.tensor.matmul(
                                    scp[:, rl:qw], kt8[:, c, kt * P:(kt + 1) * P],
                                    qt8[:, c, q0 + rl:q0 + qw],
                                    start=(c == 0), stop=(c == HT - 1))
                        if lo + P <= 0:
                            nc.scalar.activation(out=attnT[:, kt, :qw],
                                                 in_=scp[:, :qw], func=AF.Exp,
                                                 bias=ln16_t[:], scale=SCALE)
                        else:
                            if lo > 0:
                                nc.vector.memset(attnT[:, kt, 0:lo], 0.0)
                            d0 = max(lo, 0)
                            d1 = min(lo + P, qw)
                            dt_ = lay.tile([P, P], BF16, tag="dtmp", bufs=1,
                                           name="dtmp")
                            nc.scalar.activation(out=dt_[:, :d1 - d0],
                                                 in_=scp[:, d0:d1], func=AF.Exp,
                                                 bias=ln16_t[:], scale=SCALE)
                            nc.vector.tensor_mul(attnT[:, kt, d0:d1],
                                                 dt_[:, :d1 - d0],
                                                 cmask[:, d0 - lo:d1 - lo])
                            if d1 < qw:
                                nc.scalar.activation(out=attnT[:, kt, d1:qw],
                                                     in_=scp[:, d1:qw], func=AF.Exp,
                                                     bias=ln16_t[:], scale=SCALE)
                    # denominator
                    dnp = gps.tile([P, 512], F32, tag="mm", bufs=4, name="dnp",
                                   space="PSUM")
                    if ATTN_FP8:
                        for kt in range(0, ktn, 2):
                            nc.tensor.matmul(dnp[:, :qw], ones_8,
                                             attnT[:, kt:kt + 2, :qw],
                                             start=(kt == 0), stop=(kt == ktn - 2),
                                             perf_mode=DRMODE)
                    else:
                        for kt in range(ktn):
                            nc.tensor.matmul(dnp[:, :qw], ones_b,
                                             attnT[:, kt, :qw],
                                             start=(kt == 0), stop=(kt == ktn - 1))
                    dnl = lay.tile([P, 512], F32, tag="wot", bufs=2, name="dnl")
                    nc.scalar.activation(out=dnl[:, :qw], in_=dnp[:, :qw],
                                         func=AF.Ln)
                    dn = lay.tile([P, 512], BF16, tag="dn", bufs=1, name="dn")
                    nc.scalar.activation(out=dn[:, :qw], in_=dnl[:, :qw],
                                         func=AF.Exp, scale=-1.0)
                    # AV
                    ot = lay.tile([P, HT, 512], BF16, tag="ot", bufs=1, name="ot")
                    for m in range(HT):
                        avp = gps.tile([P, 512], F32, tag="mm", bufs=4,
                                       name="avp", space="PSUM")
                        if ATTN_FP8:
                            for kt in range(0, ktn, 2):
                                nc.tensor.matmul(
                                    avp[:, :qw], vn8[:, kt:kt + 2, m * P:(m + 1) * P],
                                    attnT[:, kt:kt + 2, :qw],
                                    start=(kt == 0), stop=(kt == ktn - 2),
                                    perf_mode=DRMODE)
                        else:
                            for kt in range(ktn):
                                nc.tensor.matmul(
                                    avp[:, :qw], vn8[:, kt, m * P:(m + 1) * P],
                                    attnT[:, kt, :qw],
                                    start=(kt == 0), stop=(kt == ktn - 1))
                        nc.vector.tensor_copy(ot[:, m, :qw], avp[:, :qw])
                    # Wo + deferred softmax-normalization + residual
                    for m2 in range(HT):
                        wop = gps.tile([P, 512], F32, tag="mm", bufs=4,
                                       name="wop", space="PSUM")
                        for k in range(HT):
                            nc.tensor.matmul(wop[:, :qw],
                                             wo_sb[:, k, m2 * P:(m2 + 1) * P],
                                             ot[:, k, :qw],
                                             start=(k == 0), stop=(k == HT - 1))
                        wot = lay.tile([P, 512], F32, tag="wot", bufs=2,
                                       name="wot")
                        nc.vector.tensor_mul(wot[:, :qw], wop[:, :qw], dn[:, :qw])
                        nc.vector.scalar_tensor_tensor(
                            out=xT[:, m2, q0:q0 + qw], in0=wot[:, :qw],
                            scalar=ppp[:, _BO + m2:_BO + m2 + 1],
                            in1=xT[:, m2, q0:q0 + qw], op0=OP.add, op1=OP.add)
                    # hoist LN2 stats as soon as their columns are final
                    if iqb == 0:
                        stats2(0)
                    elif iqb == 1 and len(f_cks) > 1:
                        stats2(1)
                lw_att.release()

                w2_sb = lw_ffn.tile([P, FT, H], BF16, tag="w2", name="w2")
                nc.sync.dma_start(out=w2_sb, in_=w2_x[l].rearrange("c p j -> p c j"))

                # ===== FFN =====
                g1 = lay.tile([P, FT, 384], BF16, tag="g1", name="g1")
                for ci, (c0, cw) in enumerate(f_cks):
                    hc = pend2.pop(ci)
                    for m in range(FT):
                        f1p = gps.tile([P, 512], F32, tag="mm", bufs=4,
                                       name="f1p", space="PSUM")
                        for k in range(HT):
                            nc.tensor.matmul(
                                f1p[:, :cw], w1_sb[:, k, m * P:(m + 1) * P],
                                hc[:, k, :cw],
                                start=(k == 0), stop=(k == HT - 1))
                        nc.scalar.activation(out=g1[:, m, :cw], in_=f1p[:, :cw],
                                             func=AF.Gelu,
                                             bias=ppp[:, _B1 + m:_B1 + m + 1])
                    for m2 in range(HT):
                        f2p = gps.tile([P, 512], F32, tag="mm", bufs=4,
                                       name="f2p", space="PSUM")
                        for k in range(FT):
                            nc.tensor.matmul(
                                f2p[:, :cw], w2_sb[:, k, m2 * P:(m2 + 1) * P],
                                g1[:, k, :cw],
                                start=(k == 0), stop=(k == FT - 1))
                        nc.vector.scalar_tensor_tensor(
                            out=xT[:, m2, c0:c0 + cw], in0=f2p[:, :cw],
                            scalar=ppp[:, _B2 + m2:_B2 + m2 + 1],
                            in1=xT[:, m2, c0:c0 + cw], op0=OP.add, op1=OP.add)
                    if ci + 2 < len(f_cks):
                        stats2(ci + 2)
                # LN1 stats of next layer for chunks 0/1 (residual final)
                if l + 1 < L:
                    stats1(l + 1, 0)
                    stats1(l + 1, 1)
                lw_ffn.release()
            gps.release()

            # ---- final LN on last column + classifier (f32 small ops) ----
            with tc.tile_pool(name="head", bufs=1) as sb, \
                 tc.tile_pool(name="headp", bufs=1, space="PSUM") as ps:
                col = S - 1
                xcol = sb.tile([P, HT, 1], F32, tag="hxcol")
                nc.vector.tensor_copy(xcol, xT[:, :, col:col + 1])
                ones_f = sb.tile([P, P], F32, tag="hones")
                nc.vector.memset(ones_f, 1.0)
                s1 = ps.tile([P, 1], F32, tag="hp", bufs=2, space="PSUM")
                for c in range(HT):
                    nc.tensor.matmul(s1, ones_f, xcol[:, c, :],
                                     start=(c == 0), stop=(c == HT - 1))
                sqc = sb.tile([P, HT, 1], F32, tag="hsq")
                nc.vector.tensor_mul(sqc, xcol, xcol)
                s2 = ps.tile([P, 1], F32, tag="hp", bufs=2, space="PSUM")
                for c in range(HT):
                    nc.tensor.matmul(s2, ones_f, sqc[:, c, :],
                                     start=(c == 0), stop=(c == HT - 1))
                mean = sb.tile([P, 1], F32, tag="hmean")
                nc.vector.tensor_scalar_mul(mean, s1, 1.0 / H)
                rstd = sb.tile([P, 1], F32, tag="hrstd")
                nc.vector.tensor_scalar_mul(rstd, s2, 1.0 / H)
                m2_ = sb.tile([P, 1], F32, tag="hm2")
                nc.vector.tensor_mul(m2_, mean, mean)
                nc.vector.tensor_sub(rstd, rstd, m2_)
                nc.scalar.activation(out=rstd, in_=rstd, func=AF.Sqrt,
                                     bias=eps_t[:])
                nc.vector.reciprocal(rstd, rstd)
                nf = sb.tile([P, HT, 1], F32, tag="hnf")
                for c in range(HT):
                    t = sb.tile([P, 1], F32, tag="ht", bufs=2)
                    nc.vector.tensor_sub(t, xcol[:, c, :], mean)
                    nc.vector.tensor_mul(t, t, rstd)
                    nc.vector.tensor_scalar(
                        out=nf[:, c, :], in0=t,
                        scalar1=fpp[:, c:c + 1], scalar2=fpp[:, 6 + c:7 + c],
                        op0=OP.mult, op1=OP.add)
                lp = ps.tile([P, 1], F32, tag="hp", bufs=2, space="PSUM")
                for c in range(HT):
                    nc.tensor.matmul(lp[0:1, :], nf[:, c, :], fpp[:, 12 + c:13 + c],
                                     start=(c == 0), stop=(c == HT - 1))
                cb = sb.tile([P, 1], F32, tag="hcb")
                nc.sync.dma_start(out=cb[0:1, :], in_=clsb_x[:])
                oo = sb.tile([P, 1], F32, tag="hoo")
                nc.vector.tensor_add(oo[0:1, :], lp[0:1, :], cb[0:1, :])
                nc.sync.dma_start(out=out_x[:], in_=oo[0:1, :])

    nc.finalize()
    return nc


def _pack_host(inputs, S, L):
    f32 = np.float32
    bf16 = ml_dtypes.bfloat16

    def npf(x):
        return np.asarray(x, dtype=f32)

    Wq, Wk, Wv = npf(inputs["Wq"]), npf(inputs["Wk"]), npf(inputs["Wv"])
    Wo, W1, W2 = npf(inputs["Wo"]), npf(inputs["W1"]), npf(inputs["W2"])
    g1 = npf(inputs["ln1_g"])[:, :, None]
    b1 = npf(inputs["ln1_b"])
    g2 = npf(inputs["ln2_g"])[:, :, None]
    b2 = npf(inputs["ln2_b"])

    Wq_e, Wk_e, Wv_e = g1 * Wq, g1 * Wk, g1 * Wv
    W1_e = g2 * W1

    bq_e = np.einsum('lh,lhj->lj', b1, Wq) + npf(inputs["bq"])
    bv_e = np.einsum('lh,lhj->lj', b1, Wv) + npf(inputs["bv"])
    bo_e = npf(inputs["bo"]) + np.einsum('lh,lhj->lj', bv_e, Wo)
    b1_e = np.einsum('lh,lhj->lj', b2, W1) + npf(inputs["b1"])
    b2_e = npf(inputs["b2"])

    fp8 = ml_dtypes.float8_e4m3fn
    wqk8 = np.clip(np.stack([Wq_e, Wk_e], axis=1) * 64.0, -240, 240)
    wqk8 = wqk8.reshape(L, 2, HT, P, H).astype(fp8)
    wv_p = Wv_e.reshape(L, HT, P, H).astype(bf16)
    wo_p = Wo.reshape(L, HT, P, H).astype(bf16)
    w1_p = W1_e.reshape(L, HT, P, FF).astype(bf16)
    w2_p = W2.reshape(L, FT, P, H).astype(bf16)

    def pcol(v, n):  # [L, n*128] -> [L, 128, n]
        return np.ascontiguousarray(np.transpose(v.reshape(L, n, P), (0, 2, 1)))

    ppp = np.concatenate([
        pcol(bq_e, HT), pcol(bo_e, HT), pcol(b2_e, HT), pcol(b1_e, FT),
    ], axis=2)
    assert ppp.shape == (L, P, _PPPW)

    fpp = np.concatenate([
        npf(inputs["fln_g"]).reshape(HT, P).T,
        npf(inputs["fln_b"]).reshape(HT, P).T,
        npf(inputs["cls_W"]).reshape(HT, P).T,
    ], axis=1)

    cm = (np.arange(P)[None, :] >= np.arange(P)[:, None])  # [k, q] valid q>=k

    return {
        "tok_emb": npf(inputs["tok_emb"]),
        "pos": npf(inputs["pos_emb"])[:S],
        "ident": np.eye(P, dtype=f32),
        "cmask": cm.astype(bf16),
        "wqk8": wqk8, "wv_p": wv_p, "wo_p": wo_p, "w1_p": w1_p, "w2_p": w2_p,
        "ppp": ppp,
        "fpp": np.ascontiguousarray(fpp),
        "clsb": npf(inputs["cls_b"]).reshape(1, 1),
    }


_NC_CACHE = {}


def run_model(inputs, S=S_FULL, L=L_FULL, B=B_FULL, q_last=True, trace=False):
    mask = np.asarray(inputs["attention_mask"])
    if not np.all(mask == 1):
        raise NotImplementedError("padded attention_mask not supported")

    shared = _pack_host(inputs, S, L)
    ids = np.asarray(inputs["input_ids"]).astype(np.int32)  # [B, S]
    in_maps = []
    for b in range(B):
        m = dict(shared)
        m["ids32"] = np.ascontiguousarray(ids[b].reshape(S // P, P, 1))
        in_maps.append(m)

    key = (S, L, q_last)
    if key not in _NC_CACHE:
        _NC_CACHE[key] = build_nc(S, L, q_last)
    nc = _NC_CACHE[key]

    res = run_bass_kernel_spmd(nc, in_maps, list(range(B)), trace=trace)
    out = np.stack([res.results[b]["out"].reshape(1) for b in range(B)], axis=0)
    return out.astype(np.float32), res


def kernel(**inputs) -> np.ndarray:
    out, _ = run_model(inputs, S=S_FULL, L=L_FULL, B=B_FULL)
    return out
